# revision 5
# baseline (speedup 1.0000x reference)
"""Trainium2 Bass kernel for gnn_message_passing (nn_CMP_71236327571847).

Distribution: nodes sharded 8 ways (250 nodes/core). Message passing runs as
indirect-DMA gathers + segment-sum matmuls; the conv encoder runs per node-pair
as block-diagonal [96,96] fp32r shifted-window matmuls. Train-mode BatchNorm
stats are reduced across cores with 4 AllReduces inside one SPMD launch.

Self-contained: everything (shapes, sharding) is hardcoded for the V=2000,
C=16, H=W=32, E=4000 problem; a cfg dict allows scaled-down self-tests.
"""
import sys
import numpy as np

sys.path.insert(0, "/opt/trn_rl_repo")

import concourse.bass as bass
import concourse.bacc as bacc
import concourse.tile as tile
import concourse.mybir as mybir
from concourse import bass_utils
from concourse.bass import IndirectOffsetOnAxis

F32 = mybir.dt.float32
F32R = mybir.dt.float32r
U32 = mybir.dt.uint32
AX = mybir.AxisListType.X
ALU = mybir.AluOpType
ACTF = mybir.ActivationFunctionType

EPS = 1e-5
N_CORES = 8

FULL_CFG = dict(V=2000, C=16, S=1024, NB=3)  # NB: 128-edge gather batches per group


def _derived(cfg):
    V, C, S, NB = cfg["V"], cfg["C"], cfg["S"], cfg["NB"]
    Vc = V // N_CORES          # nodes per core
    NP = Vc // 2               # node pairs per core
    NH = Vc // 2               # nodes per half (pass-A group)
    C3 = 3 * C                 # 48
    CP = 2 * C3                # 96 partition rows (2 nodes)
    W_PAD = 34                 # padded width (1+32+1)
    FLAT = W_PAD * W_PAD       # 1156
    XP_F = FLAT + 4            # padded-tile free size (AP slack for tail reads)
    PSW = 32 * W_PAD           # conv psum width 1088 (32 rows x 34)
    CHW = C * S                # 16384
    NCH = 2                    # channels per pass-A chunk
    CHK = NCH * S              # 2048
    NCK = C // NCH             # 8 chunks
    NG = 4                     # pass-A groups: sign*2 + half
    return dict(V=V, C=C, S=S, NB=NB, Vc=Vc, NP=NP, NH=NH, C3=C3, CP=CP,
                W_PAD=W_PAD, FLAT=FLAT, XP_F=XP_F, PSW=PSW, CHW=CHW,
                NCH=NCH, CHK=CHK, NCK=NCK, NG=NG, invN=1.0 / (V * S))


_PROGRAM_CACHE = {}


def _build_program(cfg):
    key = tuple(sorted(cfg.items()))
    if key in _PROGRAM_CACHE:
        return _PROGRAM_CACHE[key]
    d = _derived(cfg)
    V, C, S, NB = d["V"], d["C"], d["S"], d["NB"]
    Vc, NP, NH, C3, CP = d["Vc"], d["NP"], d["NH"], d["C3"], d["CP"]
    W_PAD, FLAT, XP_F, PSW, CHW = d["W_PAD"], d["FLAT"], d["XP_F"], d["PSW"], d["CHW"]
    NCH, CHK, NCK, NG, invN = d["NCH"], d["CHK"], d["NCK"], d["NG"], d["invN"]
    NGB = NG * NB

    nc = bacc.Bacc("TRN2", target_bir_lowering=False, debug=False,
                   enable_asserts=True, num_devices=N_CORES)

    feats_d = nc.dram_tensor("feats", [V, CHW], F32, kind="ExternalInput").ap()
    fown_d = nc.dram_tensor("feats_own", [Vc, C, S], F32, kind="ExternalInput").ap()
    idx_d = nc.dram_tensor("gidx", [128, NGB], U32, kind="ExternalInput").ap()
    smat_d = nc.dram_tensor("smat", [128, NGB * NH], F32, kind="ExternalInput").ap()
    w_d = {k: nc.dram_tensor(f"w{k}", [CP, 9 * CP], F32, kind="ExternalInput").ap()
           for k in ("1a", "1b", "2a", "2b")}
    wf_d = nc.dram_tensor("wf", [CP, 9 * 2 * C], F32, kind="ExternalInput").ap()
    bnp_d = nc.dram_tensor("bnp", [CP, 12], F32, kind="ExternalInput").ap()
    zz_d = nc.dram_tensor("zz", [128, CHK], F32, kind="ExternalInput").ap()
    eye_d = nc.dram_tensor("eye", [128, 128], F32, kind="ExternalInput").ap()
    out_d = nc.dram_tensor("out", [Vc, C, S], F32, kind="ExternalOutput").ap()

    # bnp columns
    G1A, B1A, G1B, B1B, G2A, B2A, G2B, B2B, CB1A, CB1B, CB2A, CB2B = range(12)

    with tile.TileContext(nc) as tc:
        with tc.tile_pool(name="persist", bufs=1) as pp, \
             tc.tile_pool(name="dram", bufs=1, space="DRAM") as drp:

            # ---------- persistent tiles ----------
            w_s = {k: pp.tile([CP, 9 * CP], F32R, name=f"w{k}_s") for k in w_d}
            for k in w_d:
                nc.sync.dma_start(w_s[k][:], w_d[k][:].bitcast(F32R))
            wf_s = pp.tile([CP, 9 * 2 * C], F32R)
            nc.sync.dma_start(wf_s[:], wf_d[:].bitcast(F32R))
            bnp_s = pp.tile([CP, 12], F32)
            nc.sync.dma_start(bnp_s[:], bnp_d[:])
            idx_s = pp.tile([128, NGB], U32)
            nc.sync.dma_start(idx_s[:], idx_d[:])
            smat_s = pp.tile([128, NGB * NH], F32R)
            nc.sync.dma_start(smat_s[:], smat_d[:].bitcast(F32R))
            eye_s = pp.tile([128, 128], F32)
            nc.sync.dma_start(eye_s[:], eye_d[:])

            # padded conv-input ring (pads stay zero forever)
            N_XP = 4
            xp_ring = []
            for i in range(N_XP):
                t = pp.tile([CP, XP_F], F32R, name=f"xp{i}")
                nc.sync.dma_start(t[:], zz_d[0:CP, 0:XP_F].bitcast(F32R))
                xp_ring.append(t)

            # gather ring (zero-filled once; OOB pad rows then keep stale finite data)
            N_G = 3
            g_ring = []
            for i in range(N_G):
                t = pp.tile([128, CHK], F32R, name=f"gbuf{i}")
                nc.sync.dma_start(t[:], zz_d[:, 0:CHK].bitcast(F32R))
                g_ring.append(t)

            # DRAM scratch
            pooled_d = drp.tile([2, Vc, C, S], F32)
            h1_d = drp.tile([Vc, C3, S], F32)
            x1_d = drp.tile([Vc, C3, S], F32)
            h3_d = drp.tile([Vc, C3, S], F32)

            # stat wide buffers for segments B/C/D (sum, sumsq)
            stw = {}
            for seg in ("B", "C", "D"):
                a = pp.tile([CP, 128], F32, name=f"stw{seg}_sum")
                b = pp.tile([CP, 128], F32, name=f"stw{seg}_sq")
                nc.vector.memset(a[:], 0.0)
                nc.vector.memset(b[:], 0.0)
                stw[seg] = (a, b)

            # pass-A channel-stat partials: [NH, 96] (cols: 48 sums + 48 sumsq)
            pa_s = pp.tile([NH, 2 * C3], F32)
            nc.vector.memset(pa_s[:], 0.0)

            # ---------- helpers ----------
            def bn_scale_bias(sv, parts_layout, gcol, bcol, tag):
                """sv: SBUF stats; parts_layout: True -> [96,1] with sums at
                partitions 0:48 and sumsq at 48:96; False -> [96,2] cols
                (sum, sq) with per-half partials to fold. Returns [CP,1]
                scale & bias tiles."""
                with tc.tile_pool(name=f"bnsb_{tag}", bufs=1) as bp:
                    if parts_layout:
                        sums = sv[0:C3, 0:1]
                        msrc = bp.tile([C3, 1], F32, name=f"ms_{tag}")
                        nc.sync.dma_start(msrc[:], sv[C3:CP, 0:1])
                        sq = msrc[:]
                    else:
                        tmp = bp.tile([C3, 2], F32, name=f"tmp_{tag}")
                        nc.sync.dma_start(tmp[:], sv[C3:CP, :])
                        tot = bp.tile([C3, 2], F32, name=f"tot_{tag}")
                        nc.vector.tensor_add(tot[:], sv[0:C3, :], tmp[:])
                        sums = tot[:, 0:1]
                        sq = tot[:, 1:2]
                    mean = bp.tile([C3, 1], F32, name=f"mean_{tag}")
                    nc.vector.tensor_scalar_mul(mean[:], sums, invN)
                    msq = bp.tile([C3, 1], F32, name=f"msq_{tag}")
                    nc.vector.tensor_scalar_mul(msq[:], sq, invN)
                    var = bp.tile([C3, 1], F32, name=f"var_{tag}")
                    # var = msq - mean*mean  ==  (mean * -mean) + msq
                    nc.vector.scalar_tensor_tensor(
                        var[:], mean[:], -1.0, mean[:], ALU.mult, ALU.mult)
                    nc.vector.tensor_add(var[:], var[:], msq[:])
                    nc.vector.tensor_scalar_add(var[:], var[:], EPS)
                    rec = bp.tile([C3, 1], F32, name=f"rec_{tag}")
                    nc.vector.reciprocal(rec[:], var[:])
                    rstd = bp.tile([C3, 1], F32, name=f"rstd_{tag}")
                    nc.scalar.activation(rstd[:], rec[:], ACTF.Sqrt)
                    sc48 = bp.tile([C3, 1], F32, name=f"sc48_{tag}")
                    nc.vector.tensor_mul(sc48[:], rstd[:], bnp_s[0:C3, gcol:gcol + 1])
                    # bias = b - mean*scale = (mean * -scale) + b
                    bi48 = bp.tile([C3, 1], F32, name=f"bi48_{tag}")
                    nc.vector.tensor_mul(bi48[:], mean[:], sc48[:])
                    nc.vector.scalar_tensor_tensor(
                        bi48[:], bi48[:], -1.0, bnp_s[0:C3, bcol:bcol + 1],
                        ALU.mult, ALU.add)
                    sc96 = pp.tile([CP, 1], F32, name=f"sc96_{tag}")
                    bi96 = pp.tile([CP, 1], F32, name=f"bi96_{tag}")
                    nc.sync.dma_start(sc96[0:C3, :], sc48[:])
                    nc.sync.dma_start(sc96[C3:CP, :], sc48[:])
                    nc.sync.dma_start(bi96[0:C3, :], bi48[:])
                    nc.sync.dma_start(bi96[C3:CP, :], bi48[:])
                return sc96, bi96

            def allreduce(sv_sb, tag):
                """AllReduce an SBUF stats tile across all cores (in place shape)."""
                shape = list(sv_sb.shape)
                ar_in = drp.tile(shape, F32, name=f"arin_{tag}")
                ar_out = drp.tile(shape, F32, name=f"arout_{tag}",
                                  addr_space="Shared")
                nc.sync.dma_start(ar_in[:], sv_sb[:])
                nc.gpsimd.collective_compute(
                    "AllReduce", ALU.add,
                    replica_groups=[list(range(N_CORES))],
                    ins=[ar_in[:]], outs=[ar_out[:]])
                post = pp.tile(shape, F32, name=f"arpost_{tag}")
                nc.sync.dma_start(post[:], ar_out[:])
                return post

            def conv_mms(psum_t, w_tile, mcols, xp_flat, start_fresh=True):
                """9-tap accumulating conv matmuls into psum_t[:, 0:PSW]."""
                chunks = [(0, 512), (512, 512), (1024, PSW - 1024)]
                for (j0, ln) in chunks:
                    for t in range(9):
                        di, dj = t // 3, t % 3
                        s0 = j0 + di * W_PAD + dj
                        nc.tensor.matmul(
                            psum_t[:, j0:j0 + ln],
                            w_tile[:, mcols * t:mcols * (t + 1)],
                            xp_flat[:, s0:s0 + ln],
                            start=(t == 0 and start_fresh), stop=(t == 8))

            # ---------- pass A: gather + segment-sum + stats ----------
            with tc.tile_pool(name="psA", bufs=1, space="PSUM") as psA, \
                 tc.tile_pool(name="psT", bufs=1, space="PSUM") as psT, \
                 tc.tile_pool(name="workA", bufs=2) as wa:
                git = 0
                for g in range(NG):
                    sign, half = g // 2, g % 2
                    for c in range(NCK):
                        ps = psA.tile([NH, CHK], F32, name="psa", tag="psa")
                        for b in range(NB):
                            gb = g * NB + b
                            gt = g_ring[git % N_G]
                            git += 1
                            nc.gpsimd.indirect_dma_start(
                                out=gt[:], out_offset=None,
                                in_=feats_d[:].bitcast(F32R),
                                in_offset=IndirectOffsetOnAxis(
                                    ap=idx_s[:, gb:gb + 1], axis=0),
                                element_offset=c * CHK,
                                bounds_check=V - 1, oob_is_err=False)
                            for q in range(CHK // 512):
                                nc.tensor.matmul(
                                    ps[:, q * 512:(q + 1) * 512],
                                    smat_s[:, gb * NH:(gb + 1) * NH],
                                    gt[:, q * 512:(q + 1) * 512],
                                    start=(b == 0), stop=(b == NB - 1))
                        # pooled out (psum -> sbuf -> HBM; DMA can't read PSUM)
                        cpy = wa.tile([NH, CHK], F32, name="cpyA", tag="cpyA")
                        nc.vector.tensor_copy(cpy[:], ps[:])
                        nc.sync.dma_start(
                            pooled_d[sign, half * NH:(half + 1) * NH,
                                     NCH * c:NCH * (c + 1), :],
                            cpy[:])
                        # stats: sum + sumsq per channel into pa_s columns
                        ps3 = cpy.rearrange("p (c s) -> p c s", s=S)
                        red = wa.tile([NH, NCH], F32, name="redA", tag="redA")
                        nc.vector.reduce_sum(red[:], ps3, axis=AX)
                        col = C * (1 + sign) + NCH * c
                        nc.vector.tensor_add(
                            pa_s[:, col:col + NCH], pa_s[:, col:col + NCH], red[:])
                        sqa = wa.tile([NH, CHK], F32, name="sqA", tag="sqA")
                        nc.scalar.activation(sqa[:], cpy[:], ACTF.Square)
                        red2 = wa.tile([NH, NCH], F32, name="redA2", tag="redA2")
                        nc.vector.reduce_sum(
                            red2[:], sqa.rearrange("p (c s) -> p c s", s=S), axis=AX)
                        nc.vector.tensor_add(
                            pa_s[:, C3 + col:C3 + col + NCH],
                            pa_s[:, C3 + col:C3 + col + NCH], red2[:])

                # own-feats stats (channel cols 0:C)
                for h in range(2):
                    for c in range(NCK):
                        ft = wa.tile([NH, CHK], F32, name="fownt", tag="fownt")
                        nc.sync.dma_start(
                            ft[:],
                            fown_d[h * NH:(h + 1) * NH,
                                   NCH * c:NCH * (c + 1), :])
                        ft3 = ft.rearrange("p (c s) -> p c s", s=S)
                        red = wa.tile([NH, NCH], F32, name="redA", tag="redA")
                        nc.vector.reduce_sum(red[:], ft3, axis=AX)
                        col = NCH * c
                        nc.vector.tensor_add(
                            pa_s[:, col:col + NCH], pa_s[:, col:col + NCH], red[:])
                        sqa = wa.tile([NH, CHK], F32, name="sqA", tag="sqA")
                        nc.scalar.activation(sqa[:], ft[:], ACTF.Square)
                        red2 = wa.tile([NH, NCH], F32, name="redA2", tag="redA2")
                        nc.vector.reduce_sum(
                            red2[:], sqa.rearrange("p (c s) -> p c s", s=S), axis=AX)
                        nc.vector.tensor_add(
                            pa_s[:, C3 + col:C3 + col + NCH],
                            pa_s[:, C3 + col:C3 + col + NCH], red2[:])

                # partition-reduce: transpose [NH, 96] -> [96, NH], then free reduce
                pst = psT.tile([2 * C3, NH], F32)
                nc.tensor.transpose(pst[:], pa_s[:], eye_s[0:NH, 0:NH])
                sA = pp.tile([CP, 1], F32, name="sA")
                nc.vector.reduce_sum(sA[:], pst[:], axis=AX)

            sA_post = allreduce(sA, "bn1a")
            sc1a, bi1a = bn_scale_bias(sA_post, True, G1A, B1A, "bn1a")

            # ---------- segments B..E ----------
            with tc.tile_pool(name="psC", bufs=2, space="PSUM") as psC, \
                 tc.tile_pool(name="work", bufs=3) as wk:

                def load_x_parts(dst, v, interior):
                    """Load [feats|pp|pn] for pair v into dst.
                    interior: dst is a padded [CP,34,34] f32r tile (write
                    interior), else a flat [CP, S] f32 tile."""
                    cast = (lambda ap: ap.bitcast(F32R)) if interior else (lambda ap: ap)
                    for n in range(2):
                        node = 2 * v + n
                        po = C3 * n
                        if interior:
                            tgt = lambda a, b: dst[a:b, 1:33, 1:33]
                        else:
                            tgt = lambda a, b: dst[a:b, :].rearrange(
                                "p (a b) -> p a b", b=32)
                        nc.sync.dma_start(
                            tgt(po, po + C),
                            cast(fown_d[node].rearrange("c (a b) -> c a b", b=32)))
                        nc.sync.dma_start(
                            tgt(po + C, po + 2 * C),
                            cast(pooled_d[0, node].rearrange("c (a b) -> c a b", b=32)))
                        nc.sync.dma_start(
                            tgt(po + 2 * C, po + 3 * C),
                            cast(pooled_d[1, node].rearrange("c (a b) -> c a b", b=32)))

                # ======== segment B: h1 = conv1a(relu(bn1a(x))) + b1a ========
                for v in range(NP):
                    xp = xp_ring[v % N_XP]
                    xp3 = xp[:, 0:FLAT].rearrange("p (a b) -> p a b", b=W_PAD)
                    load_x_parts(xp3, v, True)
                    itr = xp3[:, 1:33, 1:33]
                    nc.scalar.activation(itr, itr.bitcast(F32), ACTF.Relu,
                                         bias=bi1a[:], scale=sc1a[:])
                    ps = psC.tile([CP, PSW], F32, name="cps", tag="cps")
                    conv_mms(ps, w_s["1a"], CP, xp[:])
                    ps_int = ps.rearrange("p (r q) -> p r q", q=W_PAD)[:, :, 0:32]
                    h1t = wk.tile([CP, S], F32, name="h1t", tag="hseg")
                    nc.vector.tensor_scalar(
                        h1t[:].rearrange("p (r q) -> p r q", q=32), ps_int,
                        bnp_s[:, CB1A:CB1A + 1], 0.0, ALU.add, ALU.add,
                        accum_out=stw["B"][0][:, v:v + 1])
                    sqt = wk.tile([CP, S], F32, name="sqt", tag="sqseg")
                    nc.scalar.activation(sqt[:], h1t[:], ACTF.Square,
                                         accum_out=stw["B"][1][:, v:v + 1])
                    nc.sync.dma_start(h1_d[2 * v:2 * v + 2], h1t[:])

                svB = pp.tile([CP, 2], F32, name="svB")
                nc.vector.reduce_sum(svB[:, 0:1], stw["B"][0][:], axis=AX)
                nc.vector.reduce_sum(svB[:, 1:2], stw["B"][1][:], axis=AX)
                svB_post = allreduce(svB, "bn1b")
                sc1b, bi1b = bn_scale_bias(svB_post, False, G1B, B1B, "bn1b")

                # ======== segment C: x1 = x + conv1b(relu(bn1b(h1))) + b1b ========
                for v in range(NP):
                    xp = xp_ring[v % N_XP]
                    xp3 = xp[:, 0:FLAT].rearrange("p (a b) -> p a b", b=W_PAD)
                    nc.sync.dma_start(
                        xp3[:, 1:33, 1:33],
                        h1_d[2 * v:2 * v + 2].rearrange(
                            "n c (a b) -> (n c) a b", b=32).bitcast(F32R))
                    itr = xp3[:, 1:33, 1:33]
                    nc.scalar.activation(itr, itr.bitcast(F32), ACTF.Relu,
                                         bias=bi1b[:], scale=sc1b[:])
                    ps = psC.tile([CP, PSW], F32, name="cps", tag="cps")
                    conv_mms(ps, w_s["1b"], CP, xp[:])
                    xf = wk.tile([CP, S], F32, name="xf", tag="xfseg")
                    load_x_parts(xf, v, False)
                    ps_int = ps.rearrange("p (r q) -> p r q", q=W_PAD)[:, :, 0:32]
                    x1t = wk.tile([CP, S], F32, name="x1t", tag="hseg")
                    nc.vector.scalar_tensor_tensor(
                        x1t[:].rearrange("p (r q) -> p r q", q=32), ps_int,
                        bnp_s[:, CB1B:CB1B + 1],
                        xf[:].rearrange("p (r q) -> p r q", q=32),
                        ALU.add, ALU.add,
                        accum_out=stw["C"][0][:, v:v + 1])
                    sqt = wk.tile([CP, S], F32, name="sqt", tag="sqseg")
                    nc.scalar.activation(sqt[:], x1t[:], ACTF.Square,
                                         accum_out=stw["C"][1][:, v:v + 1])
                    nc.sync.dma_start(x1_d[2 * v:2 * v + 2], x1t[:])

                svC = pp.tile([CP, 2], F32, name="svC")
                nc.vector.reduce_sum(svC[:, 0:1], stw["C"][0][:], axis=AX)
                nc.vector.reduce_sum(svC[:, 1:2], stw["C"][1][:], axis=AX)
                svC_post = allreduce(svC, "bn2a")
                sc2a, bi2a = bn_scale_bias(svC_post, False, G2A, B2A, "bn2a")

                # ======== segment D: h3 = conv2a(relu(bn2a(x1))) + b2a ========
                for v in range(NP):
                    xp = xp_ring[v % N_XP]
                    xp3 = xp[:, 0:FLAT].rearrange("p (a b) -> p a b", b=W_PAD)
                    nc.sync.dma_start(
                        xp3[:, 1:33, 1:33],
                        x1_d[2 * v:2 * v + 2].rearrange(
                            "n c (a b) -> (n c) a b", b=32).bitcast(F32R))
                    itr = xp3[:, 1:33, 1:33]
                    nc.scalar.activation(itr, itr.bitcast(F32), ACTF.Relu,
                                         bias=bi2a[:], scale=sc2a[:])
                    ps = psC.tile([CP, PSW], F32, name="cps", tag="cps")
                    conv_mms(ps, w_s["2a"], CP, xp[:])
                    ps_int = ps.rearrange("p (r q) -> p r q", q=W_PAD)[:, :, 0:32]
                    h3t = wk.tile([CP, S], F32, name="h3t", tag="hseg")
                    nc.vector.tensor_scalar(
                        h3t[:].rearrange("p (r q) -> p r q", q=32), ps_int,
                        bnp_s[:, CB2A:CB2A + 1], 0.0, ALU.add, ALU.add,
                        accum_out=stw["D"][0][:, v:v + 1])
                    sqt = wk.tile([CP, S], F32, name="sqt", tag="sqseg")
                    nc.scalar.activation(sqt[:], h3t[:], ACTF.Square,
                                         accum_out=stw["D"][1][:, v:v + 1])
                    nc.sync.dma_start(h3_d[2 * v:2 * v + 2], h3t[:])

                svD = pp.tile([CP, 2], F32, name="svD")
                nc.vector.reduce_sum(svD[:, 0:1], stw["D"][0][:], axis=AX)
                nc.vector.reduce_sum(svD[:, 1:2], stw["D"][1][:], axis=AX)
                svD_post = allreduce(svD, "bn2b")
                sc2b, bi2b = bn_scale_bias(svD_post, False, G2B, B2B, "bn2b")

                # ======== segment E: x2 = x1 + conv2b(relu(bn2b(h3))) + b2b;
                #          out = relu(instnorm(convf(x2))) ========
                inv_s = 1.0 / S
                for v in range(NP):
                    xp = xp_ring[(2 * v) % N_XP]
                    xp3 = xp[:, 0:FLAT].rearrange("p (a b) -> p a b", b=W_PAD)
                    nc.sync.dma_start(
                        xp3[:, 1:33, 1:33],
                        h3_d[2 * v:2 * v + 2].rearrange(
                            "n c (a b) -> (n c) a b", b=32).bitcast(F32R))
                    itr = xp3[:, 1:33, 1:33]
                    nc.scalar.activation(itr, itr.bitcast(F32), ACTF.Relu,
                                         bias=bi2b[:], scale=sc2b[:])
                    ps = psC.tile([CP, PSW], F32, name="cps", tag="cps")
                    conv_mms(ps, w_s["2b"], CP, xp[:])
                    x1f = wk.tile([CP, S], F32, name="x1f", tag="xfseg")
                    nc.sync.dma_start(x1f[:], x1_d[2 * v:2 * v + 2])
                    # x2 into padded tile (f32r) for convf
                    xq = xp_ring[(2 * v + 1) % N_XP]
                    xq3 = xq[:, 0:FLAT].rearrange("p (a b) -> p a b", b=W_PAD)
                    ps_int = ps.rearrange("p (r q) -> p r q", q=W_PAD)[:, :, 0:32]
                    nc.vector.scalar_tensor_tensor(
                        xq3[:, 1:33, 1:33], ps_int,
                        bnp_s[:, CB2B:CB2B + 1],
                        x1f[:].rearrange("p (r q) -> p r q", q=32),
                        ALU.add, ALU.add)
                    psy = psC.tile([2 * C, PSW], F32, name="cpsy", tag="cps")
                    conv_mms(psy, wf_s, 2 * C, xq[:])
                    # instance norm per (node, channel) partition
                    py_int = psy.rearrange("p (r q) -> p r q", q=W_PAD)[:, :, 0:32]
                    ysum = wk.tile([2 * C, 1], F32, name="ysum", tag="ysum")
                    nc.vector.reduce_sum(ysum[:], py_int, axis=mybir.AxisListType.XY)
                    ysq = wk.tile([2 * C, S], F32, name="ysq", tag="ysq")
                    ysqs = wk.tile([2 * C, 1], F32, name="ysqs", tag="ysqs")
                    nc.scalar.activation(ysq[:].rearrange("p (r q) -> p r q", q=32),
                                         py_int, ACTF.Square, accum_out=ysqs[:])
                    ym = wk.tile([2 * C, 1], F32, name="ym", tag="ym")
                    nc.vector.tensor_scalar_mul(ym[:], ysum[:], inv_s)
                    yv = wk.tile([2 * C, 1], F32, name="yv", tag="yv")
                    # var = ysqs/S - ym^2 = (ym * -ym) + ysqs/S
                    nc.vector.tensor_scalar_mul(yv[:], ysqs[:], inv_s)
                    ym2 = wk.tile([2 * C, 1], F32, name="ym2", tag="ym2")
                    nc.vector.tensor_mul(ym2[:], ym[:], ym[:])
                    nc.vector.tensor_sub(yv[:], yv[:], ym2[:])
                    nc.vector.tensor_scalar_add(yv[:], yv[:], EPS)
                    yrc = wk.tile([2 * C, 1], F32, name="yrc", tag="yrc")
                    nc.vector.reciprocal(yrc[:], yv[:])
                    yr = wk.tile([2 * C, 1], F32, name="yr", tag="yr")
                    nc.scalar.activation(yr[:], yrc[:], ACTF.Sqrt)
                    ynb = wk.tile([2 * C, 1], F32, name="ynb", tag="ynb")
                    nc.vector.tensor_mul(ynb[:], ym[:], yr[:])
                    nc.vector.tensor_scalar_mul(ynb[:], ynb[:], -1.0)
                    yo = wk.tile([2 * C, S], F32, name="yo", tag="yo")
                    nc.scalar.activation(yo[:].rearrange("p (r q) -> p r q", q=32),
                                         py_int, ACTF.Relu,
                                         bias=ynb[:], scale=yr[:])
                    nc.sync.dma_start(out_d[2 * v:2 * v + 2], yo[:])

    nc.compile()
    _PROGRAM_CACHE[key] = (nc, d)
    return nc, d


def _host_prep(feats, edges, params, cfg):
    d = _derived(cfg)
    V, C, S, NB = d["V"], d["C"], d["S"], d["NB"]
    Vc, NH, C3, CP, CHW, NG = d["Vc"], d["NH"], d["C3"], d["CP"], d["CHW"], d["NG"]
    NGB = NG * NB

    feats = np.asarray(feats, np.float32).reshape(V, CHW)
    edges = np.asarray(edges)
    src, sgn, dst = edges[:, 0], edges[:, 1], edges[:, 2]
    dsrc = np.concatenate([src, dst]).astype(np.int64)
    ddst = np.concatenate([dst, src]).astype(np.int64)
    dsgn = np.concatenate([sgn, sgn])

    # per-core edge groups
    idx_all = np.full((N_CORES, 128, NGB), 60000, np.uint32)
    smat_all = np.zeros((N_CORES, 128, NGB * NH), np.float32)
    for k in range(N_CORES):
        lo = k * Vc
        m = (ddst >= lo) & (ddst < lo + Vc)
        es, ed, eg = dsrc[m], ddst[m] - lo, dsgn[m]
        for g in range(NG):
            sign, half = g // 2, g % 2
            sel = ((eg < 0) if sign else (eg > 0)) & (ed // NH == half)
            gs, gd = es[sel], ed[sel] % NH
            order = np.argsort(gd, kind="stable")
            gs, gd = gs[order], gd[order]
            ne = len(gs)
            assert ne <= NB * 128, f"core {k} group {g}: {ne} edges > {NB*128}"
            for b in range(NB):
                gb = g * NB + b
                seg = slice(b * 128, min((b + 1) * 128, ne))
                n = seg.stop - seg.start
                if n <= 0:
                    continue
                idx_all[k, :n, gb] = gs[seg]
                smat_all[k, np.arange(n), gb * NH + gd[seg]] = 1.0

    # weights: block-diag per tap
    def pack_w(wname, cout):
        W = np.asarray(params[wname], np.float32)
        out = np.zeros((CP, 9 * 2 * cout), np.float32)
        for t in range(9):
            di, dj = t // 3, t % 3
            blk = W[:, :, di, dj].T  # [Cin=48, cout]
            out[0:C3, 2 * cout * t: 2 * cout * t + cout] = blk
            out[C3:CP, 2 * cout * t + cout: 2 * cout * (t + 1)] = blk
        return out

    w_host = {k: pack_w(f"conv{k}_w", C3) for k in ("1a", "1b", "2a", "2b")}
    wf_host = pack_w("convf_w", C)

    bnp = np.zeros((CP, 12), np.float32)
    cols = [("bn1a_g", 0), ("bn1a_b", 1), ("bn1b_g", 2), ("bn1b_b", 3),
            ("bn2a_g", 4), ("bn2a_b", 5), ("bn2b_g", 6), ("bn2b_b", 7),
            ("conv1a_b", 8), ("conv1b_b", 9), ("conv2a_b", 10), ("conv2b_b", 11)]
    for name, cc in cols:
        val = np.asarray(params[name], np.float32)
        bnp[0:C3, cc] = val
        bnp[C3:CP, cc] = val

    zz = np.zeros((128, d["CHK"]), np.float32)
    eye = np.eye(128, dtype=np.float32)

    in_maps = []
    for k in range(N_CORES):
        in_maps.append({
            "feats": feats,
            "feats_own": feats[k * Vc:(k + 1) * Vc].reshape(Vc, C, S),
            "gidx": idx_all[k],
            "smat": smat_all[k],
            **{f"w{kk}": w_host[kk] for kk in w_host},
            "wf": wf_host,
            "bnp": bnp,
            "zz": zz,
            "eye": eye,
        })
    return in_maps


def run(feats, edges, params, cfg=None, trace=False):
    cfg = cfg or FULL_CFG
    d = _derived(cfg)
    nc, _ = _build_program(cfg)
    in_maps = _host_prep(feats, edges, params, cfg)
    res = bass_utils.run_bass_kernel_spmd(
        nc, in_maps, core_ids=list(range(N_CORES)), trace=trace)
    out = np.concatenate([res.results[k]["out"] for k in range(N_CORES)], axis=0)
    out = out.reshape(cfg["V"], cfg["C"], 32, 32).astype(np.float32)
    return out, res


def kernel(feats, edges, params):
    out, _ = run(feats, edges, params, FULL_CFG)
    return out


# revision 10
# speedup vs baseline: 1.0360x; 1.0360x over previous
"""Trainium2 Bass kernel for gnn_message_passing (nn_CMP_71236327571847).

Distribution: nodes sharded 8 ways (250 nodes/core). Message passing runs as
indirect-DMA gathers + segment-sum matmuls; the conv encoder runs per node-pair
as block-diagonal [96,96] fp32r shifted-window matmuls. Train-mode BatchNorm
stats are reduced across cores with 4 AllReduces inside one SPMD launch.

Self-contained: everything (shapes, sharding) is hardcoded for the V=2000,
C=16, H=W=32, E=4000 problem; a cfg dict allows scaled-down self-tests.
"""
import sys
import numpy as np

sys.path.insert(0, "/opt/trn_rl_repo")

import concourse.bass as bass
import concourse.bacc as bacc
import concourse.tile as tile
import concourse.mybir as mybir
from concourse import bass_utils
from concourse.bass import IndirectOffsetOnAxis

F32 = mybir.dt.float32
F32R = mybir.dt.float32r
U32 = mybir.dt.uint32
AX = mybir.AxisListType.X
ALU = mybir.AluOpType
ACTF = mybir.ActivationFunctionType

EPS = 1e-5
N_CORES = 8

FULL_CFG = dict(V=2000, C=16, S=1024, NB=3)  # NB: 128-edge gather batches per group


def _derived(cfg):
    V, C, S, NB = cfg["V"], cfg["C"], cfg["S"], cfg["NB"]
    Vc = V // N_CORES          # nodes per core
    NP = Vc // 2               # node pairs per core
    NH = Vc // 2               # nodes per half (pass-A group)
    C3 = 3 * C                 # 48
    CP = 2 * C3                # 96 partition rows (2 nodes)
    W_PAD = 34                 # padded width (1+32+1)
    FLAT = W_PAD * W_PAD       # 1156
    XP_F = FLAT + 4            # padded-tile free size (AP slack for tail reads)
    PSW = 32 * W_PAD           # conv psum width 1088 (32 rows x 34)
    CHW = C * S                # 16384
    NCH = 2                    # channels per pass-A chunk
    CHK = NCH * S              # 2048
    NCK = C // NCH             # 8 chunks
    NG = 4                     # pass-A groups: sign*2 + half
    return dict(V=V, C=C, S=S, NB=NB, Vc=Vc, NP=NP, NH=NH, C3=C3, CP=CP,
                W_PAD=W_PAD, FLAT=FLAT, XP_F=XP_F, PSW=PSW, CHW=CHW,
                NCH=NCH, CHK=CHK, NCK=NCK, NG=NG, invN=1.0 / (V * S))


_PROGRAM_CACHE = {}


def _build_program(cfg):
    key = tuple(sorted(cfg.items()))
    if key in _PROGRAM_CACHE:
        return _PROGRAM_CACHE[key]
    d = _derived(cfg)
    V, C, S, NB = d["V"], d["C"], d["S"], d["NB"]
    Vc, NP, NH, C3, CP = d["Vc"], d["NP"], d["NH"], d["C3"], d["CP"]
    W_PAD, FLAT, XP_F, PSW, CHW = d["W_PAD"], d["FLAT"], d["XP_F"], d["PSW"], d["CHW"]
    NCH, CHK, NCK, NG, invN = d["NCH"], d["CHK"], d["NCK"], d["NG"], d["invN"]
    NGB = NG * NB

    nc = bacc.Bacc("TRN2", target_bir_lowering=False, debug=False,
                   enable_asserts=True, num_devices=N_CORES)

    feats_d = nc.dram_tensor("feats", [V, CHW], F32, kind="ExternalInput").ap()
    fown_d = nc.dram_tensor("feats_own", [Vc, C, S], F32, kind="ExternalInput").ap()
    idx_d = nc.dram_tensor("gidx", [128, NGB], U32, kind="ExternalInput").ap()
    smat_d = nc.dram_tensor("smat", [128, NGB * NH], F32, kind="ExternalInput").ap()
    w_d = {k: nc.dram_tensor(f"w{k}", [CP, 9 * CP], F32, kind="ExternalInput").ap()
           for k in ("1a", "1b", "2a", "2b")}
    wf_d = nc.dram_tensor("wf", [CP, 9 * 2 * C], F32, kind="ExternalInput").ap()
    bnp_d = nc.dram_tensor("bnp", [CP, 12], F32, kind="ExternalInput").ap()
    zz_d = nc.dram_tensor("zz", [128, CHK], F32, kind="ExternalInput").ap()
    eye_d = nc.dram_tensor("eye", [128, 128], F32, kind="ExternalInput").ap()
    out_d = nc.dram_tensor("out", [Vc, C, S], F32, kind="ExternalOutput").ap()

    # bnp columns
    G1A, B1A, G1B, B1B, G2A, B2A, G2B, B2B, CB1A, CB1B, CB2A, CB2B = range(12)

    with tile.TileContext(nc) as tc:
        with tc.tile_pool(name="persist", bufs=1) as pp, \
             tc.tile_pool(name="dram", bufs=1, space="DRAM") as drp:

            # ---------- persistent tiles ----------
            w_s = {k: pp.tile([CP, 9 * CP], F32R, name=f"w{k}_s") for k in w_d}
            for k in w_d:
                nc.sync.dma_start(w_s[k][:], w_d[k][:].bitcast(F32R))
            wf_s = pp.tile([CP, 9 * 2 * C], F32R)
            nc.sync.dma_start(wf_s[:], wf_d[:].bitcast(F32R))
            bnp_s = pp.tile([CP, 12], F32)
            nc.sync.dma_start(bnp_s[:], bnp_d[:])
            idx_s = pp.tile([128, NGB], U32)
            nc.sync.dma_start(idx_s[:], idx_d[:])
            smat_s = pp.tile([128, NGB * NH], F32R)
            nc.sync.dma_start(smat_s[:], smat_d[:].bitcast(F32R))
            eye_s = pp.tile([128, 128], F32)
            nc.sync.dma_start(eye_s[:], eye_d[:])

            # padded conv-input ring (pads stay zero forever)
            N_XP = 4
            xp_ring = []
            for i in range(N_XP):
                t = pp.tile([CP, XP_F], F32R, name=f"xp{i}")
                nc.sync.dma_start(t[:], zz_d[0:CP, 0:XP_F].bitcast(F32R))
                xp_ring.append(t)

            # gather ring (zero-filled once; OOB pad rows then keep stale finite data)
            N_G = 3
            g_ring = []
            for i in range(N_G):
                t = pp.tile([128, CHK], F32R, name=f"gbuf{i}")
                nc.sync.dma_start(t[:], zz_d[:, 0:CHK].bitcast(F32R))
                g_ring.append(t)

            # DRAM scratch
            pooled_d = drp.tile([2, Vc, C, S], F32)
            h1_d = drp.tile([Vc, C3, S], F32)
            x1_d = drp.tile([Vc, C3, S], F32)
            h3_d = drp.tile([Vc, C3, S], F32)

            # stat wide buffers for segments B/C/D (sum, sumsq)
            stw = {}
            for seg in ("B", "C", "D"):
                a = pp.tile([CP, 128], F32, name=f"stw{seg}_sum")
                b = pp.tile([CP, 128], F32, name=f"stw{seg}_sq")
                nc.vector.memset(a[:], 0.0)
                nc.vector.memset(b[:], 0.0)
                stw[seg] = (a, b)

            # pass-A channel-stat partials: [NH, 96] (cols: 48 sums + 48 sumsq)
            pa_s = pp.tile([NH, 2 * C3], F32)
            nc.vector.memset(pa_s[:], 0.0)

            # ---------- helpers ----------
            def bn_scale_bias(sv, parts_layout, gcol, bcol, tag):
                """sv: SBUF stats; parts_layout: True -> [96,1] with sums at
                partitions 0:48 and sumsq at 48:96; False -> [96,2] cols
                (sum, sq) with per-half partials to fold. Returns [CP,1]
                scale & bias tiles."""
                with tc.tile_pool(name=f"bnsb_{tag}", bufs=1) as bp:
                    if parts_layout:
                        sums = sv[0:C3, 0:1]
                        msrc = bp.tile([C3, 1], F32, name=f"ms_{tag}")
                        nc.sync.dma_start(msrc[:], sv[C3:CP, 0:1])
                        sq = msrc[:]
                    else:
                        tmp = bp.tile([C3, 2], F32, name=f"tmp_{tag}")
                        nc.sync.dma_start(tmp[:], sv[C3:CP, :])
                        tot = bp.tile([C3, 2], F32, name=f"tot_{tag}")
                        nc.vector.tensor_add(tot[:], sv[0:C3, :], tmp[:])
                        sums = tot[:, 0:1]
                        sq = tot[:, 1:2]
                    mean = bp.tile([C3, 1], F32, name=f"mean_{tag}")
                    nc.vector.tensor_scalar_mul(mean[:], sums, invN)
                    msq = bp.tile([C3, 1], F32, name=f"msq_{tag}")
                    nc.vector.tensor_scalar_mul(msq[:], sq, invN)
                    var = bp.tile([C3, 1], F32, name=f"var_{tag}")
                    # var = msq - mean*mean  ==  (mean * -mean) + msq
                    nc.vector.scalar_tensor_tensor(
                        var[:], mean[:], -1.0, mean[:], ALU.mult, ALU.mult)
                    nc.vector.tensor_add(var[:], var[:], msq[:])
                    nc.vector.tensor_scalar_add(var[:], var[:], EPS)
                    rec = bp.tile([C3, 1], F32, name=f"rec_{tag}")
                    nc.vector.reciprocal(rec[:], var[:])
                    rstd = bp.tile([C3, 1], F32, name=f"rstd_{tag}")
                    nc.scalar.activation(rstd[:], rec[:], ACTF.Sqrt)
                    sc48 = bp.tile([C3, 1], F32, name=f"sc48_{tag}")
                    nc.vector.tensor_mul(sc48[:], rstd[:], bnp_s[0:C3, gcol:gcol + 1])
                    # bias = b - mean*scale = (mean * -scale) + b
                    bi48 = bp.tile([C3, 1], F32, name=f"bi48_{tag}")
                    nc.vector.tensor_mul(bi48[:], mean[:], sc48[:])
                    nc.vector.scalar_tensor_tensor(
                        bi48[:], bi48[:], -1.0, bnp_s[0:C3, bcol:bcol + 1],
                        ALU.mult, ALU.add)
                    sc96 = pp.tile([CP, 1], F32, name=f"sc96_{tag}")
                    bi96 = pp.tile([CP, 1], F32, name=f"bi96_{tag}")
                    nc.sync.dma_start(sc96[0:C3, :], sc48[:])
                    nc.sync.dma_start(sc96[C3:CP, :], sc48[:])
                    nc.sync.dma_start(bi96[0:C3, :], bi48[:])
                    nc.sync.dma_start(bi96[C3:CP, :], bi48[:])
                return sc96, bi96

            def allreduce(sv_sb, tag):
                """AllReduce an SBUF stats tile across all cores (in place shape)."""
                shape = list(sv_sb.shape)
                ar_in = drp.tile(shape, F32, name=f"arin_{tag}")
                ar_out = drp.tile(shape, F32, name=f"arout_{tag}",
                                  addr_space="Shared")
                nc.sync.dma_start(ar_in[:], sv_sb[:])
                nc.gpsimd.collective_compute(
                    "AllReduce", ALU.add,
                    replica_groups=[list(range(N_CORES))],
                    ins=[ar_in[:]], outs=[ar_out[:]])
                post = pp.tile(shape, F32, name=f"arpost_{tag}")
                nc.sync.dma_start(post[:], ar_out[:])
                return post

            def conv_mms(psum_t, w_tile, mcols, xp_flat, start_fresh=True):
                """9-tap accumulating conv matmuls into psum_t[:, 0:PSW]."""
                chunks = [(0, 512), (512, 512), (1024, PSW - 1024)]
                for (j0, ln) in chunks:
                    for t in range(9):
                        di, dj = t // 3, t % 3
                        s0 = j0 + di * W_PAD + dj
                        nc.tensor.matmul(
                            psum_t[:, j0:j0 + ln],
                            w_tile[:, mcols * t:mcols * (t + 1)],
                            xp_flat[:, s0:s0 + ln],
                            start=(t == 0 and start_fresh), stop=(t == 8))

            # ---------- pass A: gather + segment-sum + stats ----------
            with tc.tile_pool(name="psA", bufs=1, space="PSUM") as psA, \
                 tc.tile_pool(name="psT", bufs=1, space="PSUM") as psT, \
                 tc.tile_pool(name="workA", bufs=2) as wa:
                git = 0
                for g in range(NG):
                    sign, half = g // 2, g % 2
                    for c in range(NCK):
                        ps = psA.tile([NH, CHK], F32, name="psa", tag="psa")
                        for b in range(NB):
                            gb = g * NB + b
                            gt = g_ring[git % N_G]
                            git += 1
                            nc.gpsimd.indirect_dma_start(
                                out=gt[:], out_offset=None,
                                in_=feats_d[:].bitcast(F32R),
                                in_offset=IndirectOffsetOnAxis(
                                    ap=idx_s[:, gb:gb + 1], axis=0),
                                element_offset=c * CHK,
                                bounds_check=V - 1, oob_is_err=False)
                            for q in range(CHK // 512):
                                nc.tensor.matmul(
                                    ps[:, q * 512:(q + 1) * 512],
                                    smat_s[:, gb * NH:(gb + 1) * NH],
                                    gt[:, q * 512:(q + 1) * 512],
                                    start=(b == 0), stop=(b == NB - 1))
                        # pooled out (psum -> sbuf -> HBM; DMA can't read PSUM)
                        cpy = wa.tile([NH, CHK], F32, name="cpyA", tag="cpyA")
                        nc.vector.tensor_copy(cpy[:], ps[:])
                        nc.sync.dma_start(
                            pooled_d[sign, half * NH:(half + 1) * NH,
                                     NCH * c:NCH * (c + 1), :],
                            cpy[:])
                        # stats: sum + sumsq per channel into pa_s columns
                        ps3 = cpy.rearrange("p (c s) -> p c s", s=S)
                        red = wa.tile([NH, NCH], F32, name="redA", tag="redA")
                        nc.vector.reduce_sum(red[:], ps3, axis=AX)
                        col = C * (1 + sign) + NCH * c
                        nc.vector.tensor_add(
                            pa_s[:, col:col + NCH], pa_s[:, col:col + NCH], red[:])
                        sqa = wa.tile([NH, CHK], F32, name="sqA", tag="sqA")
                        nc.scalar.activation(sqa[:], cpy[:], ACTF.Square)
                        red2 = wa.tile([NH, NCH], F32, name="redA2", tag="redA2")
                        nc.vector.reduce_sum(
                            red2[:], sqa.rearrange("p (c s) -> p c s", s=S), axis=AX)
                        nc.vector.tensor_add(
                            pa_s[:, C3 + col:C3 + col + NCH],
                            pa_s[:, C3 + col:C3 + col + NCH], red2[:])

                # own-feats stats (channel cols 0:C)
                for h in range(2):
                    for c in range(NCK):
                        ft = wa.tile([NH, CHK], F32, name="fownt", tag="fownt")
                        nc.sync.dma_start(
                            ft[:],
                            fown_d[h * NH:(h + 1) * NH,
                                   NCH * c:NCH * (c + 1), :])
                        ft3 = ft.rearrange("p (c s) -> p c s", s=S)
                        red = wa.tile([NH, NCH], F32, name="redA", tag="redA")
                        nc.vector.reduce_sum(red[:], ft3, axis=AX)
                        col = NCH * c
                        nc.vector.tensor_add(
                            pa_s[:, col:col + NCH], pa_s[:, col:col + NCH], red[:])
                        sqa = wa.tile([NH, CHK], F32, name="sqA", tag="sqA")
                        nc.scalar.activation(sqa[:], ft[:], ACTF.Square)
                        red2 = wa.tile([NH, NCH], F32, name="redA2", tag="redA2")
                        nc.vector.reduce_sum(
                            red2[:], sqa.rearrange("p (c s) -> p c s", s=S), axis=AX)
                        nc.vector.tensor_add(
                            pa_s[:, C3 + col:C3 + col + NCH],
                            pa_s[:, C3 + col:C3 + col + NCH], red2[:])

                # partition-reduce: transpose [NH, 96] -> [96, NH], then free reduce
                pst = psT.tile([2 * C3, NH], F32)
                nc.tensor.transpose(pst[:], pa_s[:], eye_s[0:NH, 0:NH])
                sA = pp.tile([CP, 1], F32, name="sA")
                nc.vector.reduce_sum(sA[:], pst[:], axis=AX)

            sA_post = allreduce(sA, "bn1a")
            sc1a, bi1a = bn_scale_bias(sA_post, True, G1A, B1A, "bn1a")

            # ---------- segments B..E ----------
            with tc.tile_pool(name="psC", bufs=2, space="PSUM") as psC, \
                 tc.tile_pool(name="work", bufs=3) as wk:

                def load_x_parts(dst, v, interior):
                    """Load [feats|pp|pn] for pair v into dst.
                    interior: dst is a padded [CP,34,34] f32r tile (write
                    interior), else a flat [CP, S] f32 tile."""
                    cast = (lambda ap: ap.bitcast(F32R)) if interior else (lambda ap: ap)
                    for n in range(2):
                        node = 2 * v + n
                        po = C3 * n
                        if interior:
                            tgt = lambda a, b: dst[a:b, 1:33, 1:33]
                        else:
                            tgt = lambda a, b: dst[a:b, :].rearrange(
                                "p (a b) -> p a b", b=32)
                        nc.sync.dma_start(
                            tgt(po, po + C),
                            cast(fown_d[node].rearrange("c (a b) -> c a b", b=32)))
                        nc.sync.dma_start(
                            tgt(po + C, po + 2 * C),
                            cast(pooled_d[0, node].rearrange("c (a b) -> c a b", b=32)))
                        nc.sync.dma_start(
                            tgt(po + 2 * C, po + 3 * C),
                            cast(pooled_d[1, node].rearrange("c (a b) -> c a b", b=32)))

                # ======== segment B: h1 = conv1a(relu(bn1a(x))) + b1a ========
                for v in range(NP):
                    xp = xp_ring[v % N_XP]
                    xp3 = xp[:, 0:FLAT].rearrange("p (a b) -> p a b", b=W_PAD)
                    load_x_parts(xp3, v, True)
                    itr = xp3[:, 1:33, 1:33]
                    nc.scalar.activation(itr, itr.bitcast(F32), ACTF.Relu,
                                         bias=bi1a[:], scale=sc1a[:])
                    ps = psC.tile([CP, PSW], F32, name="cps", tag="cps")
                    conv_mms(ps, w_s["1a"], CP, xp[:])
                    ps_int = ps.rearrange("p (r q) -> p r q", q=W_PAD)[:, :, 0:32]
                    h1t = wk.tile([CP, S], F32, name="h1t", tag="hseg")
                    nc.vector.tensor_scalar(
                        h1t[:].rearrange("p (r q) -> p r q", q=32), ps_int,
                        bnp_s[:, CB1A:CB1A + 1], 0.0, ALU.add, ALU.add,
                        accum_out=stw["B"][0][:, v:v + 1])
                    sqt = wk.tile([CP, S], F32, name="sqt", tag="sqseg")
                    nc.vector.scalar_tensor_tensor(
                        sqt[:], h1t[:], 1.0, h1t[:], ALU.mult, ALU.mult,
                        accum_out=stw["B"][1][:, v:v + 1])
                    nc.sync.dma_start(h1_d[2 * v:2 * v + 2], h1t[:])

                svB = pp.tile([CP, 2], F32, name="svB")
                nc.vector.reduce_sum(svB[:, 0:1], stw["B"][0][:], axis=AX)
                nc.vector.reduce_sum(svB[:, 1:2], stw["B"][1][:], axis=AX)
                svB_post = allreduce(svB, "bn1b")
                sc1b, bi1b = bn_scale_bias(svB_post, False, G1B, B1B, "bn1b")

                # ======== segment C: x1 = x + conv1b(relu(bn1b(h1))) + b1b ========
                for v in range(NP):
                    xp = xp_ring[v % N_XP]
                    xp3 = xp[:, 0:FLAT].rearrange("p (a b) -> p a b", b=W_PAD)
                    nc.sync.dma_start(
                        xp3[:, 1:33, 1:33],
                        h1_d[2 * v:2 * v + 2].rearrange(
                            "n c (a b) -> (n c) a b", b=32).bitcast(F32R))
                    itr = xp3[:, 1:33, 1:33]
                    nc.scalar.activation(itr, itr.bitcast(F32), ACTF.Relu,
                                         bias=bi1b[:], scale=sc1b[:])
                    ps = psC.tile([CP, PSW], F32, name="cps", tag="cps")
                    conv_mms(ps, w_s["1b"], CP, xp[:])
                    xf = wk.tile([CP, S], F32, name="xf", tag="xfseg")
                    load_x_parts(xf, v, False)
                    ps_int = ps.rearrange("p (r q) -> p r q", q=W_PAD)[:, :, 0:32]
                    x1t = wk.tile([CP, S], F32, name="x1t", tag="hseg")
                    nc.vector.scalar_tensor_tensor(
                        x1t[:].rearrange("p (r q) -> p r q", q=32), ps_int,
                        bnp_s[:, CB1B:CB1B + 1],
                        xf[:].rearrange("p (r q) -> p r q", q=32),
                        ALU.add, ALU.add,
                        accum_out=stw["C"][0][:, v:v + 1])
                    sqt = wk.tile([CP, S], F32, name="sqt", tag="sqseg")
                    nc.vector.scalar_tensor_tensor(
                        sqt[:], x1t[:], 1.0, x1t[:], ALU.mult, ALU.mult,
                        accum_out=stw["C"][1][:, v:v + 1])
                    nc.sync.dma_start(x1_d[2 * v:2 * v + 2], x1t[:])

                svC = pp.tile([CP, 2], F32, name="svC")
                nc.vector.reduce_sum(svC[:, 0:1], stw["C"][0][:], axis=AX)
                nc.vector.reduce_sum(svC[:, 1:2], stw["C"][1][:], axis=AX)
                svC_post = allreduce(svC, "bn2a")
                sc2a, bi2a = bn_scale_bias(svC_post, False, G2A, B2A, "bn2a")

                # ======== segment D: h3 = conv2a(relu(bn2a(x1))) + b2a ========
                for v in range(NP):
                    xp = xp_ring[v % N_XP]
                    xp3 = xp[:, 0:FLAT].rearrange("p (a b) -> p a b", b=W_PAD)
                    nc.sync.dma_start(
                        xp3[:, 1:33, 1:33],
                        x1_d[2 * v:2 * v + 2].rearrange(
                            "n c (a b) -> (n c) a b", b=32).bitcast(F32R))
                    itr = xp3[:, 1:33, 1:33]
                    nc.scalar.activation(itr, itr.bitcast(F32), ACTF.Relu,
                                         bias=bi2a[:], scale=sc2a[:])
                    ps = psC.tile([CP, PSW], F32, name="cps", tag="cps")
                    conv_mms(ps, w_s["2a"], CP, xp[:])
                    ps_int = ps.rearrange("p (r q) -> p r q", q=W_PAD)[:, :, 0:32]
                    h3t = wk.tile([CP, S], F32, name="h3t", tag="hseg")
                    nc.vector.tensor_scalar(
                        h3t[:].rearrange("p (r q) -> p r q", q=32), ps_int,
                        bnp_s[:, CB2A:CB2A + 1], 0.0, ALU.add, ALU.add,
                        accum_out=stw["D"][0][:, v:v + 1])
                    sqt = wk.tile([CP, S], F32, name="sqt", tag="sqseg")
                    nc.vector.scalar_tensor_tensor(
                        sqt[:], h3t[:], 1.0, h3t[:], ALU.mult, ALU.mult,
                        accum_out=stw["D"][1][:, v:v + 1])
                    nc.sync.dma_start(h3_d[2 * v:2 * v + 2], h3t[:])

                svD = pp.tile([CP, 2], F32, name="svD")
                nc.vector.reduce_sum(svD[:, 0:1], stw["D"][0][:], axis=AX)
                nc.vector.reduce_sum(svD[:, 1:2], stw["D"][1][:], axis=AX)
                svD_post = allreduce(svD, "bn2b")
                sc2b, bi2b = bn_scale_bias(svD_post, False, G2B, B2B, "bn2b")

                # ======== segment E: x2 = x1 + conv2b(relu(bn2b(h3))) + b2b;
                #          out = relu(instnorm(convf(x2))) ========
                inv_s = 1.0 / S
                pend = None  # (y_sb, yr, ynb, v) awaiting final relu+store
                def flush_pend():
                    nonlocal pend
                    if pend is None:
                        return
                    p_ysb, p_yr, p_ynb, pv = pend
                    yo = wk.tile([2 * C, S], F32, name="yo", tag="yo")
                    nc.scalar.activation(yo[:], p_ysb[:], ACTF.Relu,
                                         bias=p_ynb[:], scale=p_yr[:])
                    nc.sync.dma_start(out_d[2 * pv:2 * pv + 2], yo[:])
                    pend = None
                for v in range(NP):
                    xp = xp_ring[(2 * v) % N_XP]
                    xp3 = xp[:, 0:FLAT].rearrange("p (a b) -> p a b", b=W_PAD)
                    nc.sync.dma_start(
                        xp3[:, 1:33, 1:33],
                        h3_d[2 * v:2 * v + 2].rearrange(
                            "n c (a b) -> (n c) a b", b=32).bitcast(F32R))
                    itr = xp3[:, 1:33, 1:33]
                    nc.scalar.activation(itr, itr.bitcast(F32), ACTF.Relu,
                                         bias=bi2b[:], scale=sc2b[:])
                    flush_pend()
                    ps = psC.tile([CP, PSW], F32, name="cps", tag="cps")
                    conv_mms(ps, w_s["2b"], CP, xp[:])
                    x1f = wk.tile([CP, S], F32, name="x1f", tag="xfseg")
                    nc.sync.dma_start(x1f[:], x1_d[2 * v:2 * v + 2])
                    # x2 into padded tile (f32r) for convf
                    xq = xp_ring[(2 * v + 1) % N_XP]
                    xq3 = xq[:, 0:FLAT].rearrange("p (a b) -> p a b", b=W_PAD)
                    ps_int = ps.rearrange("p (r q) -> p r q", q=W_PAD)[:, :, 0:32]
                    nc.vector.scalar_tensor_tensor(
                        xq3[:, 1:33, 1:33], ps_int,
                        bnp_s[:, CB2B:CB2B + 1],
                        x1f[:].rearrange("p (r q) -> p r q", q=32),
                        ALU.add, ALU.add)
                    psy = psC.tile([2 * C, PSW], F32, name="cpsy", tag="cps")
                    conv_mms(psy, wf_s, 2 * C, xq[:])
                    # instance norm per (node, channel) partition
                    py_int = psy.rearrange("p (r q) -> p r q", q=W_PAD)[:, :, 0:32]
                    ysum = wk.tile([2 * C, 1], F32, name="ysum", tag="ysum")
                    y_sb = wk.tile([2 * C, S], F32, name="y_sb", tag="y_sb")
                    nc.vector.tensor_scalar(
                        y_sb[:].rearrange("p (r q) -> p r q", q=32), py_int,
                        0.0, 0.0, ALU.add, ALU.add, accum_out=ysum[:])
                    ysq = wk.tile([2 * C, S], F32, name="ysq", tag="ysq")
                    ysqs = wk.tile([2 * C, 1], F32, name="ysqs", tag="ysqs")
                    nc.vector.scalar_tensor_tensor(
                        ysq[:], y_sb[:], 1.0, y_sb[:], ALU.mult, ALU.mult,
                        accum_out=ysqs[:])
                    ym = wk.tile([2 * C, 1], F32, name="ym", tag="ym")
                    nc.vector.tensor_scalar_mul(ym[:], ysum[:], inv_s)
                    yv = wk.tile([2 * C, 1], F32, name="yv", tag="yv")
                    # var = ysqs/S - ym^2 = (ym * -ym) + ysqs/S
                    nc.vector.tensor_scalar_mul(yv[:], ysqs[:], inv_s)
                    ym2 = wk.tile([2 * C, 1], F32, name="ym2", tag="ym2")
                    nc.vector.tensor_mul(ym2[:], ym[:], ym[:])
                    nc.vector.tensor_sub(yv[:], yv[:], ym2[:])
                    nc.vector.tensor_scalar_add(yv[:], yv[:], EPS)
                    yrc = wk.tile([2 * C, 1], F32, name="yrc", tag="yrc")
                    nc.vector.reciprocal(yrc[:], yv[:])
                    yr = wk.tile([2 * C, 1], F32, name="yr", tag="yr")
                    nc.scalar.activation(yr[:], yrc[:], ACTF.Sqrt)
                    ynb = wk.tile([2 * C, 1], F32, name="ynb", tag="ynb")
                    nc.vector.tensor_mul(ynb[:], ym[:], yr[:])
                    nc.vector.tensor_scalar_mul(ynb[:], ynb[:], -1.0)
                    pend = (y_sb, yr, ynb, v)
                if pend is not None:
                    flush_pend()

    nc.compile()
    _PROGRAM_CACHE[key] = (nc, d)
    return nc, d


def _host_prep(feats, edges, params, cfg):
    d = _derived(cfg)
    V, C, S, NB = d["V"], d["C"], d["S"], d["NB"]
    Vc, NH, C3, CP, CHW, NG = d["Vc"], d["NH"], d["C3"], d["CP"], d["CHW"], d["NG"]
    NGB = NG * NB

    feats = np.asarray(feats, np.float32).reshape(V, CHW)
    edges = np.asarray(edges)
    src, sgn, dst = edges[:, 0], edges[:, 1], edges[:, 2]
    dsrc = np.concatenate([src, dst]).astype(np.int64)
    ddst = np.concatenate([dst, src]).astype(np.int64)
    dsgn = np.concatenate([sgn, sgn])

    # per-core edge groups
    idx_all = np.full((N_CORES, 128, NGB), 60000, np.uint32)
    smat_all = np.zeros((N_CORES, 128, NGB * NH), np.float32)
    for k in range(N_CORES):
        lo = k * Vc
        m = (ddst >= lo) & (ddst < lo + Vc)
        es, ed, eg = dsrc[m], ddst[m] - lo, dsgn[m]
        for g in range(NG):
            sign, half = g // 2, g % 2
            sel = ((eg < 0) if sign else (eg > 0)) & (ed // NH == half)
            gs, gd = es[sel], ed[sel] % NH
            order = np.argsort(gd, kind="stable")
            gs, gd = gs[order], gd[order]
            ne = len(gs)
            assert ne <= NB * 128, f"core {k} group {g}: {ne} edges > {NB*128}"
            for b in range(NB):
                gb = g * NB + b
                seg = slice(b * 128, min((b + 1) * 128, ne))
                n = seg.stop - seg.start
                if n <= 0:
                    continue
                idx_all[k, :n, gb] = gs[seg]
                smat_all[k, np.arange(n), gb * NH + gd[seg]] = 1.0

    # weights: block-diag per tap
    def pack_w(wname, cout):
        W = np.asarray(params[wname], np.float32)
        out = np.zeros((CP, 9 * 2 * cout), np.float32)
        for t in range(9):
            di, dj = t // 3, t % 3
            blk = W[:, :, di, dj].T  # [Cin=48, cout]
            out[0:C3, 2 * cout * t: 2 * cout * t + cout] = blk
            out[C3:CP, 2 * cout * t + cout: 2 * cout * (t + 1)] = blk
        return out

    w_host = {k: pack_w(f"conv{k}_w", C3) for k in ("1a", "1b", "2a", "2b")}
    wf_host = pack_w("convf_w", C)

    bnp = np.zeros((CP, 12), np.float32)
    cols = [("bn1a_g", 0), ("bn1a_b", 1), ("bn1b_g", 2), ("bn1b_b", 3),
            ("bn2a_g", 4), ("bn2a_b", 5), ("bn2b_g", 6), ("bn2b_b", 7),
            ("conv1a_b", 8), ("conv1b_b", 9), ("conv2a_b", 10), ("conv2b_b", 11)]
    for name, cc in cols:
        val = np.asarray(params[name], np.float32)
        bnp[0:C3, cc] = val
        bnp[C3:CP, cc] = val

    zz = np.zeros((128, d["CHK"]), np.float32)
    eye = np.eye(128, dtype=np.float32)

    in_maps = []
    for k in range(N_CORES):
        in_maps.append({
            "feats": feats,
            "feats_own": feats[k * Vc:(k + 1) * Vc].reshape(Vc, C, S),
            "gidx": idx_all[k],
            "smat": smat_all[k],
            **{f"w{kk}": w_host[kk] for kk in w_host},
            "wf": wf_host,
            "bnp": bnp,
            "zz": zz,
            "eye": eye,
        })
    return in_maps


def run(feats, edges, params, cfg=None, trace=False):
    cfg = cfg or FULL_CFG
    d = _derived(cfg)
    nc, _ = _build_program(cfg)
    in_maps = _host_prep(feats, edges, params, cfg)
    res = bass_utils.run_bass_kernel_spmd(
        nc, in_maps, core_ids=list(range(N_CORES)), trace=trace)
    out = np.concatenate([res.results[k]["out"] for k in range(N_CORES)], axis=0)
    out = out.reshape(cfg["V"], cfg["C"], 32, 32).astype(np.float32)
    return out, res


def kernel(feats, edges, params):
    out, _ = run(feats, edges, params, FULL_CFG)
    return out


# revision 11
# speedup vs baseline: 1.2618x; 1.2180x over previous
"""Trainium2 Bass kernel for gnn_message_passing (nn_CMP_71236327571847).

Distribution: nodes sharded 8 ways (250 nodes/core). Message passing runs as
indirect-DMA gathers + segment-sum matmuls; the conv encoder runs per node-pair
as block-diagonal [96,96] fp32r shifted-window matmuls. Train-mode BatchNorm
stats are reduced across cores with 4 AllReduces inside one SPMD launch.

Self-contained: everything (shapes, sharding) is hardcoded for the V=2000,
C=16, H=W=32, E=4000 problem; a cfg dict allows scaled-down self-tests.
"""
import sys
import numpy as np

sys.path.insert(0, "/opt/trn_rl_repo")

import concourse.bass as bass
import concourse.bacc as bacc
import concourse.tile as tile
import concourse.mybir as mybir
from concourse import bass_utils
from concourse.bass import IndirectOffsetOnAxis

F32 = mybir.dt.float32
F32R = mybir.dt.float32r
U32 = mybir.dt.uint32
AX = mybir.AxisListType.X
ALU = mybir.AluOpType
ACTF = mybir.ActivationFunctionType

EPS = 1e-5
N_CORES = 8

FULL_CFG = dict(V=2000, C=16, S=1024, NB=3)  # NB: 128-edge gather batches per group


def _derived(cfg):
    V, C, S, NB = cfg["V"], cfg["C"], cfg["S"], cfg["NB"]
    Vc = V // N_CORES          # nodes per core
    NP = Vc // 2               # node pairs per core
    NH = Vc // 2               # nodes per half (pass-A group)
    C3 = 3 * C                 # 48
    CP = 2 * C3                # 96 partition rows (2 nodes)
    W_PAD = 34                 # padded width (1+32+1)
    FLAT = W_PAD * W_PAD       # 1156
    XP_F = FLAT + 4            # padded-tile free size (AP slack for tail reads)
    PSW = 32 * W_PAD           # conv psum width 1088 (32 rows x 34)
    CHW = C * S                # 16384
    NCH = 2                    # channels per pass-A chunk
    CHK = NCH * S              # 2048
    NCK = C // NCH             # 8 chunks
    NG = 4                     # pass-A groups: sign*2 + half
    return dict(V=V, C=C, S=S, NB=NB, Vc=Vc, NP=NP, NH=NH, C3=C3, CP=CP,
                W_PAD=W_PAD, FLAT=FLAT, XP_F=XP_F, PSW=PSW, CHW=CHW,
                NCH=NCH, CHK=CHK, NCK=NCK, NG=NG, invN=1.0 / (V * S))


_PROGRAM_CACHE = {}


def _build_program(cfg):
    key = tuple(sorted(cfg.items()))
    if key in _PROGRAM_CACHE:
        return _PROGRAM_CACHE[key]
    d = _derived(cfg)
    V, C, S, NB = d["V"], d["C"], d["S"], d["NB"]
    Vc, NP, NH, C3, CP = d["Vc"], d["NP"], d["NH"], d["C3"], d["CP"]
    W_PAD, FLAT, XP_F, PSW, CHW = d["W_PAD"], d["FLAT"], d["XP_F"], d["PSW"], d["CHW"]
    NCH, CHK, NCK, NG, invN = d["NCH"], d["CHK"], d["NCK"], d["NG"], d["invN"]
    NGB = NG * NB

    nc = bacc.Bacc("TRN2", target_bir_lowering=False, debug=False,
                   enable_asserts=True, num_devices=N_CORES)

    feats_d = nc.dram_tensor("feats", [V, CHW], F32, kind="ExternalInput").ap()
    fown_d = nc.dram_tensor("feats_own", [Vc, C, S], F32, kind="ExternalInput").ap()
    idx_d = nc.dram_tensor("gidx", [128, NGB], U32, kind="ExternalInput").ap()
    smat_d = nc.dram_tensor("smat", [128, NGB * NH], F32, kind="ExternalInput").ap()
    w_d = {k: nc.dram_tensor(f"w{k}", [CP, 9 * CP], F32, kind="ExternalInput").ap()
           for k in ("1a", "1b", "2a", "2b")}
    wf_d = nc.dram_tensor("wf", [CP, 9 * 2 * C], F32, kind="ExternalInput").ap()
    bnp_d = nc.dram_tensor("bnp", [CP, 12], F32, kind="ExternalInput").ap()
    zz_d = nc.dram_tensor("zz", [128, CHK], F32, kind="ExternalInput").ap()
    eye_d = nc.dram_tensor("eye", [128, 128], F32, kind="ExternalInput").ap()
    out_d = nc.dram_tensor("out", [Vc, C, S], F32, kind="ExternalOutput").ap()

    # bnp columns
    G1A, B1A, G1B, B1B, G2A, B2A, G2B, B2B, CB1A, CB1B, CB2A, CB2B = range(12)

    with tile.TileContext(nc) as tc:
        with tc.tile_pool(name="persist", bufs=1) as pp, \
             tc.tile_pool(name="dram", bufs=1, space="DRAM") as drp:

            # ---------- persistent tiles ----------
            w_s = {k: pp.tile([CP, 9 * CP], F32R, name=f"w{k}_s") for k in w_d}
            for k in w_d:
                nc.sync.dma_start(w_s[k][:], w_d[k][:].bitcast(F32R))
            wf_s = pp.tile([CP, 9 * 2 * C], F32R)
            nc.sync.dma_start(wf_s[:], wf_d[:].bitcast(F32R))
            bnp_s = pp.tile([CP, 12], F32)
            nc.sync.dma_start(bnp_s[:], bnp_d[:])
            idx_s = pp.tile([128, NGB], U32)
            nc.sync.dma_start(idx_s[:], idx_d[:])
            smat_s = pp.tile([128, NGB * NH], F32R)
            nc.sync.dma_start(smat_s[:], smat_d[:].bitcast(F32R))
            eye_s = pp.tile([128, 128], F32)
            nc.sync.dma_start(eye_s[:], eye_d[:])

            # padded conv-input ring (pads stay zero forever)
            N_XP = 4
            xp_ring = []
            for i in range(N_XP):
                t = pp.tile([CP, XP_F], F32R, name=f"xp{i}")
                nc.sync.dma_start(t[:], zz_d[0:CP, 0:XP_F].bitcast(F32R))
                xp_ring.append(t)

            # gather ring (zero-filled once; OOB pad rows then keep stale finite data)
            N_G = 5
            g_ring = []
            for i in range(N_G):
                t = pp.tile([128, CHK], F32R, name=f"gbuf{i}")
                nc.sync.dma_start(t[:], zz_d[:, 0:CHK].bitcast(F32R))
                g_ring.append(t)

            # DRAM scratch
            pooled_d = drp.tile([2, Vc, C, S], F32)
            h1_d = drp.tile([Vc, C3, S], F32)
            x1_d = drp.tile([Vc, C3, S], F32)
            h3_d = drp.tile([Vc, C3, S], F32)

            # stat wide buffers for segments B/C/D (sum, sumsq)
            stw = {}
            for seg in ("B", "C", "D"):
                a = pp.tile([CP, 128], F32, name=f"stw{seg}_sum")
                b = pp.tile([CP, 128], F32, name=f"stw{seg}_sq")
                nc.vector.memset(a[:], 0.0)
                nc.vector.memset(b[:], 0.0)
                stw[seg] = (a, b)

            # pass-A channel-stat partials: [NH, 96] (cols: 48 sums + 48 sumsq)
            pa_s = pp.tile([NH, 2 * C3], F32)
            nc.vector.memset(pa_s[:], 0.0)

            # ---------- helpers ----------
            def bn_scale_bias(sv, parts_layout, gcol, bcol, tag):
                """sv: SBUF stats; parts_layout: True -> [96,1] with sums at
                partitions 0:48 and sumsq at 48:96; False -> [96,2] cols
                (sum, sq) with per-half partials to fold. Returns [CP,1]
                scale & bias tiles."""
                with tc.tile_pool(name=f"bnsb_{tag}", bufs=1) as bp:
                    if parts_layout:
                        sums = sv[0:C3, 0:1]
                        msrc = bp.tile([C3, 1], F32, name=f"ms_{tag}")
                        nc.sync.dma_start(msrc[:], sv[C3:CP, 0:1])
                        sq = msrc[:]
                    else:
                        tmp = bp.tile([C3, 2], F32, name=f"tmp_{tag}")
                        nc.sync.dma_start(tmp[:], sv[C3:CP, :])
                        tot = bp.tile([C3, 2], F32, name=f"tot_{tag}")
                        nc.vector.tensor_add(tot[:], sv[0:C3, :], tmp[:])
                        sums = tot[:, 0:1]
                        sq = tot[:, 1:2]
                    mean = bp.tile([C3, 1], F32, name=f"mean_{tag}")
                    nc.vector.tensor_scalar_mul(mean[:], sums, invN)
                    msq = bp.tile([C3, 1], F32, name=f"msq_{tag}")
                    nc.vector.tensor_scalar_mul(msq[:], sq, invN)
                    var = bp.tile([C3, 1], F32, name=f"var_{tag}")
                    # var = msq - mean*mean  ==  (mean * -mean) + msq
                    nc.vector.scalar_tensor_tensor(
                        var[:], mean[:], -1.0, mean[:], ALU.mult, ALU.mult)
                    nc.vector.tensor_add(var[:], var[:], msq[:])
                    nc.vector.tensor_scalar_add(var[:], var[:], EPS)
                    rec = bp.tile([C3, 1], F32, name=f"rec_{tag}")
                    nc.vector.reciprocal(rec[:], var[:])
                    rstd = bp.tile([C3, 1], F32, name=f"rstd_{tag}")
                    nc.scalar.activation(rstd[:], rec[:], ACTF.Sqrt)
                    sc48 = bp.tile([C3, 1], F32, name=f"sc48_{tag}")
                    nc.vector.tensor_mul(sc48[:], rstd[:], bnp_s[0:C3, gcol:gcol + 1])
                    # bias = b - mean*scale = (mean * -scale) + b
                    bi48 = bp.tile([C3, 1], F32, name=f"bi48_{tag}")
                    nc.vector.tensor_mul(bi48[:], mean[:], sc48[:])
                    nc.vector.scalar_tensor_tensor(
                        bi48[:], bi48[:], -1.0, bnp_s[0:C3, bcol:bcol + 1],
                        ALU.mult, ALU.add)
                    sc96 = pp.tile([CP, 1], F32, name=f"sc96_{tag}")
                    bi96 = pp.tile([CP, 1], F32, name=f"bi96_{tag}")
                    nc.sync.dma_start(sc96[0:C3, :], sc48[:])
                    nc.sync.dma_start(sc96[C3:CP, :], sc48[:])
                    nc.sync.dma_start(bi96[0:C3, :], bi48[:])
                    nc.sync.dma_start(bi96[C3:CP, :], bi48[:])
                return sc96, bi96

            def allreduce(sv_sb, tag):
                """AllReduce an SBUF stats tile across all cores (in place shape)."""
                shape = list(sv_sb.shape)
                ar_in = drp.tile(shape, F32, name=f"arin_{tag}")
                ar_out = drp.tile(shape, F32, name=f"arout_{tag}",
                                  addr_space="Shared")
                nc.sync.dma_start(ar_in[:], sv_sb[:])
                nc.gpsimd.collective_compute(
                    "AllReduce", ALU.add,
                    replica_groups=[list(range(N_CORES))],
                    ins=[ar_in[:]], outs=[ar_out[:]])
                post = pp.tile(shape, F32, name=f"arpost_{tag}")
                nc.sync.dma_start(post[:], ar_out[:])
                return post

            def conv_mms(psum_t, w_tile, mcols, xp_flat, start_fresh=True):
                """9-tap accumulating conv matmuls into psum_t[:, 0:PSW]."""
                chunks = [(0, 512), (512, 512), (1024, PSW - 1024)]
                for (j0, ln) in chunks:
                    for t in range(9):
                        di, dj = t // 3, t % 3
                        s0 = j0 + di * W_PAD + dj
                        nc.tensor.matmul(
                            psum_t[:, j0:j0 + ln],
                            w_tile[:, mcols * t:mcols * (t + 1)],
                            xp_flat[:, s0:s0 + ln],
                            start=(t == 0 and start_fresh), stop=(t == 8))

            # ---------- pass A: gather + segment-sum + stats ----------
            with tc.tile_pool(name="psA", bufs=1, space="PSUM") as psA, \
                 tc.tile_pool(name="psT", bufs=1, space="PSUM") as psT, \
                 tc.tile_pool(name="workA", bufs=2) as wa:
                git = 0
                for g in range(NG):
                    sign, half = g // 2, g % 2
                    for c in range(NCK):
                        ps = psA.tile([NH, CHK], F32, name="psa", tag="psa")
                        for b in range(NB):
                            gb = g * NB + b
                            gt = g_ring[git % N_G]
                            git += 1
                            nc.gpsimd.indirect_dma_start(
                                out=gt[:], out_offset=None,
                                in_=feats_d[:].bitcast(F32R),
                                in_offset=IndirectOffsetOnAxis(
                                    ap=idx_s[:, gb:gb + 1], axis=0),
                                element_offset=c * CHK,
                                bounds_check=V - 1, oob_is_err=False)
                            for q in range(CHK // 512):
                                nc.tensor.matmul(
                                    ps[:, q * 512:(q + 1) * 512],
                                    smat_s[:, gb * NH:(gb + 1) * NH],
                                    gt[:, q * 512:(q + 1) * 512],
                                    start=(b == 0), stop=(b == NB - 1))
                        # pooled out (psum -> sbuf -> HBM; DMA can't read PSUM)
                        cpy = wa.tile([NH, CHK], F32, name="cpyA", tag="cpyA")
                        nc.vector.tensor_copy(cpy[:], ps[:])
                        nc.sync.dma_start(
                            pooled_d[sign, half * NH:(half + 1) * NH,
                                     NCH * c:NCH * (c + 1), :],
                            cpy[:])
                        # stats: sum + sumsq per channel into pa_s columns
                        ps3 = cpy.rearrange("p (c s) -> p c s", s=S)
                        red = wa.tile([NH, NCH], F32, name="redA", tag="redA")
                        nc.vector.reduce_sum(red[:], ps3, axis=AX)
                        col = C * (1 + sign) + NCH * c
                        nc.vector.tensor_add(
                            pa_s[:, col:col + NCH], pa_s[:, col:col + NCH], red[:])
                        sqa = wa.tile([NH, CHK], F32, name="sqA", tag="sqA")
                        nc.scalar.activation(sqa[:], cpy[:], ACTF.Square)
                        red2 = wa.tile([NH, NCH], F32, name="redA2", tag="redA2")
                        nc.vector.reduce_sum(
                            red2[:], sqa.rearrange("p (c s) -> p c s", s=S), axis=AX)
                        nc.vector.tensor_add(
                            pa_s[:, C3 + col:C3 + col + NCH],
                            pa_s[:, C3 + col:C3 + col + NCH], red2[:])

                # own-feats stats (channel cols 0:C)
                for h in range(2):
                    for c in range(NCK):
                        ft = wa.tile([NH, CHK], F32, name="fownt", tag="fownt")
                        nc.sync.dma_start(
                            ft[:],
                            fown_d[h * NH:(h + 1) * NH,
                                   NCH * c:NCH * (c + 1), :])
                        ft3 = ft.rearrange("p (c s) -> p c s", s=S)
                        red = wa.tile([NH, NCH], F32, name="redA", tag="redA")
                        nc.vector.reduce_sum(red[:], ft3, axis=AX)
                        col = NCH * c
                        nc.vector.tensor_add(
                            pa_s[:, col:col + NCH], pa_s[:, col:col + NCH], red[:])
                        sqa = wa.tile([NH, CHK], F32, name="sqA", tag="sqA")
                        nc.scalar.activation(sqa[:], ft[:], ACTF.Square)
                        red2 = wa.tile([NH, NCH], F32, name="redA2", tag="redA2")
                        nc.vector.reduce_sum(
                            red2[:], sqa.rearrange("p (c s) -> p c s", s=S), axis=AX)
                        nc.vector.tensor_add(
                            pa_s[:, C3 + col:C3 + col + NCH],
                            pa_s[:, C3 + col:C3 + col + NCH], red2[:])

                # partition-reduce: transpose [NH, 96] -> [96, NH], then free reduce
                pst = psT.tile([2 * C3, NH], F32)
                nc.tensor.transpose(pst[:], pa_s[:], eye_s[0:NH, 0:NH])
                sA = pp.tile([CP, 1], F32, name="sA")
                nc.vector.reduce_sum(sA[:], pst[:], axis=AX)

            sA_post = allreduce(sA, "bn1a")
            sc1a, bi1a = bn_scale_bias(sA_post, True, G1A, B1A, "bn1a")

            # ---------- segments B..E ----------
            with tc.tile_pool(name="psC", bufs=2, space="PSUM") as psC, \
                 tc.tile_pool(name="work", bufs=3) as wk:

                def load_x_parts(dst, v):
                    """Load [feats|pp|pn] for pair v into flat [CP, S] f32 tile."""
                    for n in range(2):
                        node = 2 * v + n
                        po = C3 * n
                        nc.sync.dma_start(dst[po:po + C, :], fown_d[node])
                        nc.sync.dma_start(dst[po + C:po + 2 * C, :], pooled_d[0, node])
                        nc.sync.dma_start(dst[po + 2 * C:po + 3 * C, :], pooled_d[1, node])

                # ======== segment B: h1 = conv1a(relu(bn1a(x))) + b1a ========
                for v in range(NP):
                    xp = xp_ring[v % N_XP]
                    xp3 = xp[:, 0:FLAT].rearrange("p (a b) -> p a b", b=W_PAD)
                    xbf = wk.tile([CP, S], F32, name="xbf", tag="xin")
                    load_x_parts(xbf, v)
                    nc.scalar.activation(xp3[:, 1:33, 1:33],
                                         xbf[:].rearrange("p (a b) -> p a b", b=32),
                                         ACTF.Relu, bias=bi1a[:], scale=sc1a[:])
                    ps = psC.tile([CP, PSW], F32, name="cps", tag="cps")
                    conv_mms(ps, w_s["1a"], CP, xp[:])
                    ps_int = ps.rearrange("p (r q) -> p r q", q=W_PAD)[:, :, 0:32]
                    h1t = wk.tile([CP, S], F32, name="h1t", tag="hseg")
                    nc.vector.tensor_scalar(
                        h1t[:].rearrange("p (r q) -> p r q", q=32), ps_int,
                        bnp_s[:, CB1A:CB1A + 1], 0.0, ALU.add, ALU.add,
                        accum_out=stw["B"][0][:, v:v + 1])
                    sqt = wk.tile([CP, S], F32, name="sqt", tag="sqseg")
                    nc.vector.scalar_tensor_tensor(
                        sqt[:], h1t[:], 1.0, h1t[:], ALU.mult, ALU.mult,
                        accum_out=stw["B"][1][:, v:v + 1])
                    nc.sync.dma_start(h1_d[2 * v:2 * v + 2], h1t[:])

                svB = pp.tile([CP, 2], F32, name="svB")
                nc.vector.reduce_sum(svB[:, 0:1], stw["B"][0][:], axis=AX)
                nc.vector.reduce_sum(svB[:, 1:2], stw["B"][1][:], axis=AX)
                svB_post = allreduce(svB, "bn1b")
                sc1b, bi1b = bn_scale_bias(svB_post, False, G1B, B1B, "bn1b")

                # ======== segment C: x1 = x + conv1b(relu(bn1b(h1))) + b1b ========
                for v in range(NP):
                    xp = xp_ring[v % N_XP]
                    xp3 = xp[:, 0:FLAT].rearrange("p (a b) -> p a b", b=W_PAD)
                    hfl = wk.tile([CP, S], F32, name="hfl", tag="xin")
                    nc.sync.dma_start(hfl[:], h1_d[2 * v:2 * v + 2])
                    nc.scalar.activation(xp3[:, 1:33, 1:33],
                                         hfl[:].rearrange("p (a b) -> p a b", b=32),
                                         ACTF.Relu, bias=bi1b[:], scale=sc1b[:])
                    ps = psC.tile([CP, PSW], F32, name="cps", tag="cps")
                    conv_mms(ps, w_s["1b"], CP, xp[:])
                    xf = wk.tile([CP, S], F32, name="xf", tag="xfseg")
                    load_x_parts(xf, v)
                    ps_int = ps.rearrange("p (r q) -> p r q", q=W_PAD)[:, :, 0:32]
                    x1t = wk.tile([CP, S], F32, name="x1t", tag="hseg")
                    nc.vector.scalar_tensor_tensor(
                        x1t[:].rearrange("p (r q) -> p r q", q=32), ps_int,
                        bnp_s[:, CB1B:CB1B + 1],
                        xf[:].rearrange("p (r q) -> p r q", q=32),
                        ALU.add, ALU.add,
                        accum_out=stw["C"][0][:, v:v + 1])
                    sqt = wk.tile([CP, S], F32, name="sqt", tag="sqseg")
                    nc.vector.scalar_tensor_tensor(
                        sqt[:], x1t[:], 1.0, x1t[:], ALU.mult, ALU.mult,
                        accum_out=stw["C"][1][:, v:v + 1])
                    nc.sync.dma_start(x1_d[2 * v:2 * v + 2], x1t[:])

                svC = pp.tile([CP, 2], F32, name="svC")
                nc.vector.reduce_sum(svC[:, 0:1], stw["C"][0][:], axis=AX)
                nc.vector.reduce_sum(svC[:, 1:2], stw["C"][1][:], axis=AX)
                svC_post = allreduce(svC, "bn2a")
                sc2a, bi2a = bn_scale_bias(svC_post, False, G2A, B2A, "bn2a")

                # ======== segment D: h3 = conv2a(relu(bn2a(x1))) + b2a ========
                for v in range(NP):
                    xp = xp_ring[v % N_XP]
                    xp3 = xp[:, 0:FLAT].rearrange("p (a b) -> p a b", b=W_PAD)
                    hfl = wk.tile([CP, S], F32, name="hfl", tag="xin")
                    nc.sync.dma_start(hfl[:], x1_d[2 * v:2 * v + 2])
                    nc.scalar.activation(xp3[:, 1:33, 1:33],
                                         hfl[:].rearrange("p (a b) -> p a b", b=32),
                                         ACTF.Relu, bias=bi2a[:], scale=sc2a[:])
                    ps = psC.tile([CP, PSW], F32, name="cps", tag="cps")
                    conv_mms(ps, w_s["2a"], CP, xp[:])
                    ps_int = ps.rearrange("p (r q) -> p r q", q=W_PAD)[:, :, 0:32]
                    h3t = wk.tile([CP, S], F32, name="h3t", tag="hseg")
                    nc.vector.tensor_scalar(
                        h3t[:].rearrange("p (r q) -> p r q", q=32), ps_int,
                        bnp_s[:, CB2A:CB2A + 1], 0.0, ALU.add, ALU.add,
                        accum_out=stw["D"][0][:, v:v + 1])
                    sqt = wk.tile([CP, S], F32, name="sqt", tag="sqseg")
                    nc.vector.scalar_tensor_tensor(
                        sqt[:], h3t[:], 1.0, h3t[:], ALU.mult, ALU.mult,
                        accum_out=stw["D"][1][:, v:v + 1])
                    nc.sync.dma_start(h3_d[2 * v:2 * v + 2], h3t[:])

                svD = pp.tile([CP, 2], F32, name="svD")
                nc.vector.reduce_sum(svD[:, 0:1], stw["D"][0][:], axis=AX)
                nc.vector.reduce_sum(svD[:, 1:2], stw["D"][1][:], axis=AX)
                svD_post = allreduce(svD, "bn2b")
                sc2b, bi2b = bn_scale_bias(svD_post, False, G2B, B2B, "bn2b")

                # ======== segment E: x2 = x1 + conv2b(relu(bn2b(h3))) + b2b;
                #          out = relu(instnorm(convf(x2))) ========
                inv_s = 1.0 / S
                pend = None  # (y_sb, yr, ynb, v) awaiting final relu+store
                def flush_pend():
                    nonlocal pend
                    if pend is None:
                        return
                    p_ysb, p_yr, p_ynb, pv = pend
                    yo = wk.tile([2 * C, S], F32, name="yo", tag="yo")
                    nc.scalar.activation(yo[:], p_ysb[:], ACTF.Relu,
                                         bias=p_ynb[:], scale=p_yr[:])
                    nc.sync.dma_start(out_d[2 * pv:2 * pv + 2], yo[:])
                    pend = None
                for v in range(NP):
                    xp = xp_ring[(2 * v) % N_XP]
                    xp3 = xp[:, 0:FLAT].rearrange("p (a b) -> p a b", b=W_PAD)
                    hfl = wk.tile([CP, S], F32, name="hfl", tag="xin")
                    nc.sync.dma_start(hfl[:], h3_d[2 * v:2 * v + 2])
                    nc.scalar.activation(xp3[:, 1:33, 1:33],
                                         hfl[:].rearrange("p (a b) -> p a b", b=32),
                                         ACTF.Relu, bias=bi2b[:], scale=sc2b[:])
                    flush_pend()
                    ps = psC.tile([CP, PSW], F32, name="cps", tag="cps")
                    conv_mms(ps, w_s["2b"], CP, xp[:])
                    x1f = wk.tile([CP, S], F32, name="x1f", tag="xfseg")
                    nc.sync.dma_start(x1f[:], x1_d[2 * v:2 * v + 2])
                    # x2 into padded tile (f32r) for convf
                    xq = xp_ring[(2 * v + 1) % N_XP]
                    xq3 = xq[:, 0:FLAT].rearrange("p (a b) -> p a b", b=W_PAD)
                    ps_int = ps.rearrange("p (r q) -> p r q", q=W_PAD)[:, :, 0:32]
                    nc.vector.scalar_tensor_tensor(
                        xq3[:, 1:33, 1:33], ps_int,
                        bnp_s[:, CB2B:CB2B + 1],
                        x1f[:].rearrange("p (r q) -> p r q", q=32),
                        ALU.add, ALU.add)
                    psy = psC.tile([2 * C, PSW], F32, name="cpsy", tag="cps")
                    conv_mms(psy, wf_s, 2 * C, xq[:])
                    # instance norm per (node, channel) partition
                    py_int = psy.rearrange("p (r q) -> p r q", q=W_PAD)[:, :, 0:32]
                    ysum = wk.tile([2 * C, 1], F32, name="ysum", tag="ysum")
                    y_sb = wk.tile([2 * C, S], F32, name="y_sb", tag="y_sb")
                    nc.vector.tensor_scalar(
                        y_sb[:].rearrange("p (r q) -> p r q", q=32), py_int,
                        0.0, 0.0, ALU.add, ALU.add, accum_out=ysum[:])
                    ysq = wk.tile([2 * C, S], F32, name="ysq", tag="ysq")
                    ysqs = wk.tile([2 * C, 1], F32, name="ysqs", tag="ysqs")
                    nc.vector.scalar_tensor_tensor(
                        ysq[:], y_sb[:], 1.0, y_sb[:], ALU.mult, ALU.mult,
                        accum_out=ysqs[:])
                    ym = wk.tile([2 * C, 1], F32, name="ym", tag="ym")
                    nc.vector.tensor_scalar_mul(ym[:], ysum[:], inv_s)
                    yv = wk.tile([2 * C, 1], F32, name="yv", tag="yv")
                    # var = ysqs/S - ym^2 = (ym * -ym) + ysqs/S
                    nc.vector.tensor_scalar_mul(yv[:], ysqs[:], inv_s)
                    ym2 = wk.tile([2 * C, 1], F32, name="ym2", tag="ym2")
                    nc.vector.tensor_mul(ym2[:], ym[:], ym[:])
                    nc.vector.tensor_sub(yv[:], yv[:], ym2[:])
                    nc.vector.tensor_scalar_add(yv[:], yv[:], EPS)
                    yrc = wk.tile([2 * C, 1], F32, name="yrc", tag="yrc")
                    nc.vector.reciprocal(yrc[:], yv[:])
                    yr = wk.tile([2 * C, 1], F32, name="yr", tag="yr")
                    nc.scalar.activation(yr[:], yrc[:], ACTF.Sqrt)
                    ynb = wk.tile([2 * C, 1], F32, name="ynb", tag="ynb")
                    nc.vector.tensor_mul(ynb[:], ym[:], yr[:])
                    nc.vector.tensor_scalar_mul(ynb[:], ynb[:], -1.0)
                    pend = (y_sb, yr, ynb, v)
                if pend is not None:
                    flush_pend()

    nc.compile()
    _PROGRAM_CACHE[key] = (nc, d)
    return nc, d


def _host_prep(feats, edges, params, cfg):
    d = _derived(cfg)
    V, C, S, NB = d["V"], d["C"], d["S"], d["NB"]
    Vc, NH, C3, CP, CHW, NG = d["Vc"], d["NH"], d["C3"], d["CP"], d["CHW"], d["NG"]
    NGB = NG * NB

    feats = np.asarray(feats, np.float32).reshape(V, CHW)
    edges = np.asarray(edges)
    src, sgn, dst = edges[:, 0], edges[:, 1], edges[:, 2]
    dsrc = np.concatenate([src, dst]).astype(np.int64)
    ddst = np.concatenate([dst, src]).astype(np.int64)
    dsgn = np.concatenate([sgn, sgn])

    # per-core edge groups
    idx_all = np.full((N_CORES, 128, NGB), 60000, np.uint32)
    smat_all = np.zeros((N_CORES, 128, NGB * NH), np.float32)
    for k in range(N_CORES):
        lo = k * Vc
        m = (ddst >= lo) & (ddst < lo + Vc)
        es, ed, eg = dsrc[m], ddst[m] - lo, dsgn[m]
        for g in range(NG):
            sign, half = g // 2, g % 2
            sel = ((eg < 0) if sign else (eg > 0)) & (ed // NH == half)
            gs, gd = es[sel], ed[sel] % NH
            order = np.argsort(gd, kind="stable")
            gs, gd = gs[order], gd[order]
            ne = len(gs)
            assert ne <= NB * 128, f"core {k} group {g}: {ne} edges > {NB*128}"
            for b in range(NB):
                gb = g * NB + b
                seg = slice(b * 128, min((b + 1) * 128, ne))
                n = seg.stop - seg.start
                if n <= 0:
                    continue
                idx_all[k, :n, gb] = gs[seg]
                smat_all[k, np.arange(n), gb * NH + gd[seg]] = 1.0

    # weights: block-diag per tap
    def pack_w(wname, cout):
        W = np.asarray(params[wname], np.float32)
        out = np.zeros((CP, 9 * 2 * cout), np.float32)
        for t in range(9):
            di, dj = t // 3, t % 3
            blk = W[:, :, di, dj].T  # [Cin=48, cout]
            out[0:C3, 2 * cout * t: 2 * cout * t + cout] = blk
            out[C3:CP, 2 * cout * t + cout: 2 * cout * (t + 1)] = blk
        return out

    w_host = {k: pack_w(f"conv{k}_w", C3) for k in ("1a", "1b", "2a", "2b")}
    wf_host = pack_w("convf_w", C)

    bnp = np.zeros((CP, 12), np.float32)
    cols = [("bn1a_g", 0), ("bn1a_b", 1), ("bn1b_g", 2), ("bn1b_b", 3),
            ("bn2a_g", 4), ("bn2a_b", 5), ("bn2b_g", 6), ("bn2b_b", 7),
            ("conv1a_b", 8), ("conv1b_b", 9), ("conv2a_b", 10), ("conv2b_b", 11)]
    for name, cc in cols:
        val = np.asarray(params[name], np.float32)
        bnp[0:C3, cc] = val
        bnp[C3:CP, cc] = val

    zz = np.zeros((128, d["CHK"]), np.float32)
    eye = np.eye(128, dtype=np.float32)

    in_maps = []
    for k in range(N_CORES):
        in_maps.append({
            "feats": feats,
            "feats_own": feats[k * Vc:(k + 1) * Vc].reshape(Vc, C, S),
            "gidx": idx_all[k],
            "smat": smat_all[k],
            **{f"w{kk}": w_host[kk] for kk in w_host},
            "wf": wf_host,
            "bnp": bnp,
            "zz": zz,
            "eye": eye,
        })
    return in_maps


def run(feats, edges, params, cfg=None, trace=False):
    cfg = cfg or FULL_CFG
    d = _derived(cfg)
    nc, _ = _build_program(cfg)
    in_maps = _host_prep(feats, edges, params, cfg)
    res = bass_utils.run_bass_kernel_spmd(
        nc, in_maps, core_ids=list(range(N_CORES)), trace=trace)
    out = np.concatenate([res.results[k]["out"] for k in range(N_CORES)], axis=0)
    out = out.reshape(cfg["V"], cfg["C"], 32, 32).astype(np.float32)
    return out, res


def kernel(feats, edges, params):
    out, _ = run(feats, edges, params, FULL_CFG)
    return out


# revision 13
# speedup vs baseline: 1.7153x; 1.3595x over previous
"""Trainium2 Bass kernel for gnn_message_passing (nn_CMP_71236327571847).

Distribution: nodes sharded 8 ways (250 nodes/core). Message passing runs as
indirect-DMA gathers + segment-sum matmuls; the conv encoder runs per node-pair
as block-diagonal [96,96] fp32r shifted-window matmuls. Train-mode BatchNorm
stats are reduced across cores with 4 AllReduces inside one SPMD launch.

Self-contained: everything (shapes, sharding) is hardcoded for the V=2000,
C=16, H=W=32, E=4000 problem; a cfg dict allows scaled-down self-tests.
"""
import sys
import numpy as np

sys.path.insert(0, "/opt/trn_rl_repo")

import concourse.bass as bass
import concourse.bacc as bacc
import concourse.tile as tile
import concourse.mybir as mybir
from concourse import bass_utils
from concourse.bass import IndirectOffsetOnAxis

F32 = mybir.dt.float32
F32R = mybir.dt.float32r
U32 = mybir.dt.uint32
AX = mybir.AxisListType.X
ALU = mybir.AluOpType
ACTF = mybir.ActivationFunctionType

EPS = 1e-5
N_CORES = 8

FULL_CFG = dict(V=2000, C=16, S=1024, NB=3)  # NB: 128-edge gather batches per group


def _derived(cfg):
    V, C, S, NB = cfg["V"], cfg["C"], cfg["S"], cfg["NB"]
    Vc = V // N_CORES          # nodes per core
    NP = Vc // 2               # node pairs per core
    NH = Vc // 2               # nodes per half (pass-A group)
    C3 = 3 * C                 # 48
    CP = 2 * C3                # 96 partition rows (2 nodes)
    W_PAD = 34                 # padded width (1+32+1)
    FLAT = W_PAD * W_PAD       # 1156
    XP_F = FLAT + 4            # padded-tile free size (AP slack for tail reads)
    PSW = 32 * W_PAD           # conv psum width 1088 (32 rows x 34)
    CHW = C * S                # 16384
    NCH = 2                    # channels per pass-A chunk
    CHK = NCH * S              # 2048
    NCK = C // NCH             # 8 chunks
    NG = 4                     # pass-A groups: sign*2 + half
    return dict(V=V, C=C, S=S, NB=NB, Vc=Vc, NP=NP, NH=NH, C3=C3, CP=CP,
                W_PAD=W_PAD, FLAT=FLAT, XP_F=XP_F, PSW=PSW, CHW=CHW,
                NCH=NCH, CHK=CHK, NCK=NCK, NG=NG, invN=1.0 / (V * S))


_PROGRAM_CACHE = {}


def _build_program(cfg):
    key = tuple(sorted(cfg.items()))
    if key in _PROGRAM_CACHE:
        return _PROGRAM_CACHE[key]
    d = _derived(cfg)
    V, C, S, NB = d["V"], d["C"], d["S"], d["NB"]
    Vc, NP, NH, C3, CP = d["Vc"], d["NP"], d["NH"], d["C3"], d["CP"]
    W_PAD, FLAT, XP_F, PSW, CHW = d["W_PAD"], d["FLAT"], d["XP_F"], d["PSW"], d["CHW"]
    NCH, CHK, NCK, NG, invN = d["NCH"], d["CHK"], d["NCK"], d["NG"], d["invN"]
    NGB = NG * NB

    nc = bacc.Bacc("TRN2", target_bir_lowering=False, debug=False,
                   enable_asserts=True, num_devices=N_CORES)

    feats_d = nc.dram_tensor("feats", [V, CHW], F32, kind="ExternalInput").ap()
    fown_d = nc.dram_tensor("feats_own", [Vc, C, S], F32, kind="ExternalInput").ap()
    idx_d = nc.dram_tensor("gidx", [128, NGB], U32, kind="ExternalInput").ap()
    smat_d = nc.dram_tensor("smat", [128, NGB * NH], F32, kind="ExternalInput").ap()
    w_d = {k: nc.dram_tensor(f"w{k}", [CP, 9 * CP], F32, kind="ExternalInput").ap()
           for k in ("1a", "1b", "2a", "2b")}
    wf_d = nc.dram_tensor("wf", [CP, 9 * 2 * C], F32, kind="ExternalInput").ap()
    bnp_d = nc.dram_tensor("bnp", [CP, 12], F32, kind="ExternalInput").ap()
    zz_d = nc.dram_tensor("zz", [128, CHK], F32, kind="ExternalInput").ap()
    eye_d = nc.dram_tensor("eye", [128, 128], F32, kind="ExternalInput").ap()
    out_d = nc.dram_tensor("out", [Vc, C, S], F32, kind="ExternalOutput").ap()

    # bnp columns
    G1A, B1A, G1B, B1B, G2A, B2A, G2B, B2B, CB1A, CB1B, CB2A, CB2B = range(12)

    with tile.TileContext(nc) as tc:
        with tc.tile_pool(name="persist", bufs=1) as pp, \
             tc.tile_pool(name="dram", bufs=1, space="DRAM") as drp:

            # ---------- persistent tiles ----------
            w_s = {k: pp.tile([CP, 9 * CP], F32R, name=f"w{k}_s") for k in w_d}
            for k in w_d:
                nc.sync.dma_start(w_s[k][:], w_d[k][:].bitcast(F32R))
            wf_s = pp.tile([CP, 9 * 2 * C], F32R)
            nc.sync.dma_start(wf_s[:], wf_d[:].bitcast(F32R))
            bnp_s = pp.tile([CP, 12], F32)
            nc.sync.dma_start(bnp_s[:], bnp_d[:])
            idx_s = pp.tile([128, NGB], U32)
            nc.sync.dma_start(idx_s[:], idx_d[:])
            smat_s = pp.tile([128, NGB * NH], F32R)
            nc.sync.dma_start(smat_s[:], smat_d[:].bitcast(F32R))
            eye_s = pp.tile([128, 128], F32)
            nc.sync.dma_start(eye_s[:], eye_d[:])

            # padded conv-input ring (pads stay zero forever)
            N_XP = 4
            xp_ring = []
            for i in range(N_XP):
                t = pp.tile([CP, XP_F], F32R, name=f"xp{i}")
                nc.sync.dma_start(t[:], zz_d[0:CP, 0:XP_F].bitcast(F32R))
                xp_ring.append(t)

            # gather ring (zero-filled once; OOB pad rows then keep stale finite data)
            N_G = 5
            g_ring = []
            for i in range(N_G):
                t = pp.tile([128, CHK], F32R, name=f"gbuf{i}")
                nc.sync.dma_start(t[:], zz_d[:, 0:CHK].bitcast(F32R))
                g_ring.append(t)

            # DRAM scratch
            pooled_d = drp.tile([2, Vc, C, S], F32)
            h1_d = drp.tile([Vc, C3, S], F32)
            x1_d = drp.tile([Vc, C3, S], F32)
            h3_d = drp.tile([Vc, C3, S], F32)

            # stat wide buffers for segments B/C/D (sum, sumsq)
            stw = {}
            for seg in ("B", "C", "D"):
                a = pp.tile([CP, 128], F32, name=f"stw{seg}_sum")
                b = pp.tile([CP, 128], F32, name=f"stw{seg}_sq")
                nc.vector.memset(a[:], 0.0)
                nc.vector.memset(b[:], 0.0)
                stw[seg] = (a, b)

            # pass-A channel-stat partials: [NH, 96] (cols: 48 sums + 48 sumsq)
            pa_s = pp.tile([NH, 2 * C3], F32)
            nc.vector.memset(pa_s[:], 0.0)

            # ---------- helpers ----------
            def bn_scale_bias(sv, parts_layout, gcol, bcol, tag):
                """sv: SBUF stats; parts_layout: True -> [96,1] with sums at
                partitions 0:48 and sumsq at 48:96; False -> [96,2] cols
                (sum, sq) with per-half partials to fold. Returns [CP,1]
                scale & bias tiles."""
                with tc.tile_pool(name=f"bnsb_{tag}", bufs=1) as bp:
                    if parts_layout:
                        sums = sv[0:C3, 0:1]
                        msrc = bp.tile([C3, 1], F32, name=f"ms_{tag}")
                        nc.sync.dma_start(msrc[:], sv[C3:CP, 0:1])
                        sq = msrc[:]
                    else:
                        tmp = bp.tile([C3, 2], F32, name=f"tmp_{tag}")
                        nc.sync.dma_start(tmp[:], sv[C3:CP, :])
                        tot = bp.tile([C3, 2], F32, name=f"tot_{tag}")
                        nc.vector.tensor_add(tot[:], sv[0:C3, :], tmp[:])
                        sums = tot[:, 0:1]
                        sq = tot[:, 1:2]
                    mean = bp.tile([C3, 1], F32, name=f"mean_{tag}")
                    nc.vector.tensor_scalar_mul(mean[:], sums, invN)
                    msq = bp.tile([C3, 1], F32, name=f"msq_{tag}")
                    nc.vector.tensor_scalar_mul(msq[:], sq, invN)
                    var = bp.tile([C3, 1], F32, name=f"var_{tag}")
                    # var = msq - mean*mean  ==  (mean * -mean) + msq
                    nc.vector.scalar_tensor_tensor(
                        var[:], mean[:], -1.0, mean[:], ALU.mult, ALU.mult)
                    nc.vector.tensor_add(var[:], var[:], msq[:])
                    nc.vector.tensor_scalar_add(var[:], var[:], EPS)
                    rec = bp.tile([C3, 1], F32, name=f"rec_{tag}")
                    nc.vector.reciprocal(rec[:], var[:])
                    rstd = bp.tile([C3, 1], F32, name=f"rstd_{tag}")
                    nc.scalar.activation(rstd[:], rec[:], ACTF.Sqrt)
                    sc48 = bp.tile([C3, 1], F32, name=f"sc48_{tag}")
                    nc.vector.tensor_mul(sc48[:], rstd[:], bnp_s[0:C3, gcol:gcol + 1])
                    # bias = b - mean*scale = (mean * -scale) + b
                    bi48 = bp.tile([C3, 1], F32, name=f"bi48_{tag}")
                    nc.vector.tensor_mul(bi48[:], mean[:], sc48[:])
                    nc.vector.scalar_tensor_tensor(
                        bi48[:], bi48[:], -1.0, bnp_s[0:C3, bcol:bcol + 1],
                        ALU.mult, ALU.add)
                    sc96 = pp.tile([CP, 1], F32, name=f"sc96_{tag}")
                    bi96 = pp.tile([CP, 1], F32, name=f"bi96_{tag}")
                    nc.sync.dma_start(sc96[0:C3, :], sc48[:])
                    nc.sync.dma_start(sc96[C3:CP, :], sc48[:])
                    nc.sync.dma_start(bi96[0:C3, :], bi48[:])
                    nc.sync.dma_start(bi96[C3:CP, :], bi48[:])
                return sc96, bi96

            def allreduce(sv_sb, tag):
                """AllReduce an SBUF stats tile across all cores (in place shape)."""
                shape = list(sv_sb.shape)
                ar_in = drp.tile(shape, F32, name=f"arin_{tag}")
                ar_out = drp.tile(shape, F32, name=f"arout_{tag}",
                                  addr_space="Shared")
                nc.sync.dma_start(ar_in[:], sv_sb[:])
                nc.gpsimd.collective_compute(
                    "AllReduce", ALU.add,
                    replica_groups=[list(range(N_CORES))],
                    ins=[ar_in[:]], outs=[ar_out[:]])
                post = pp.tile(shape, F32, name=f"arpost_{tag}")
                nc.sync.dma_start(post[:], ar_out[:])
                return post

            def conv_mms(psum_t, w_tile, mcols, xp_flat, start_fresh=True):
                """9-tap accumulating conv matmuls into psum_t[:, 0:PSW]."""
                chunks = [(0, 512), (512, 512), (1024, PSW - 1024)]
                for (j0, ln) in chunks:
                    for t in range(9):
                        di, dj = t // 3, t % 3
                        s0 = j0 + di * W_PAD + dj
                        nc.tensor.matmul(
                            psum_t[:, j0:j0 + ln],
                            w_tile[:, mcols * t:mcols * (t + 1)],
                            xp_flat[:, s0:s0 + ln],
                            start=(t == 0 and start_fresh), stop=(t == 8))

            # ---------- pass A: gather + segment-sum + stats ----------
            with tc.tile_pool(name="psA", bufs=1, space="PSUM") as psA, \
                 tc.tile_pool(name="psT", bufs=1, space="PSUM") as psT, \
                 tc.tile_pool(name="workA", bufs=2) as wa:
                git = 0
                for g in range(NG):
                    sign, half = g // 2, g % 2
                    for c in range(NCK):
                        ps = psA.tile([NH, CHK], F32, name="psa", tag="psa")
                        for b in range(NB):
                            gb = g * NB + b
                            gt = g_ring[git % N_G]
                            git += 1
                            nc.gpsimd.indirect_dma_start(
                                out=gt[:], out_offset=None,
                                in_=feats_d[:].bitcast(F32R),
                                in_offset=IndirectOffsetOnAxis(
                                    ap=idx_s[:, gb:gb + 1], axis=0),
                                element_offset=c * CHK,
                                bounds_check=V - 1, oob_is_err=False)
                            for q in range(CHK // 512):
                                nc.tensor.matmul(
                                    ps[:, q * 512:(q + 1) * 512],
                                    smat_s[:, gb * NH:(gb + 1) * NH],
                                    gt[:, q * 512:(q + 1) * 512],
                                    start=(b == 0), stop=(b == NB - 1))
                        # pooled out (psum -> sbuf -> HBM; DMA can't read PSUM)
                        cpy = wa.tile([NH, CHK], F32, name="cpyA", tag="cpyA")
                        nc.vector.tensor_copy(cpy[:], ps[:])
                        nc.gpsimd.dma_start(
                            pooled_d[sign, half * NH:(half + 1) * NH,
                                     NCH * c:NCH * (c + 1), :],
                            cpy[:])
                        # stats: sum + sumsq per channel into pa_s columns
                        ps3 = cpy.rearrange("p (c s) -> p c s", s=S)
                        red = wa.tile([NH, NCH], F32, name="redA", tag="redA")
                        nc.vector.reduce_sum(red[:], ps3, axis=AX)
                        col = C * (1 + sign) + NCH * c
                        nc.vector.tensor_add(
                            pa_s[:, col:col + NCH], pa_s[:, col:col + NCH], red[:])
                        sqa = wa.tile([NH, CHK], F32, name="sqA", tag="sqA")
                        nc.scalar.activation(sqa[:], cpy[:], ACTF.Square)
                        red2 = wa.tile([NH, NCH], F32, name="redA2", tag="redA2")
                        nc.vector.reduce_sum(
                            red2[:], sqa.rearrange("p (c s) -> p c s", s=S), axis=AX)
                        nc.vector.tensor_add(
                            pa_s[:, C3 + col:C3 + col + NCH],
                            pa_s[:, C3 + col:C3 + col + NCH], red2[:])

                # own-feats stats (channel cols 0:C)
                for h in range(2):
                    for c in range(NCK):
                        ft = wa.tile([NH, CHK], F32, name="fownt", tag="fownt")
                        nc.sync.dma_start(
                            ft[:],
                            fown_d[h * NH:(h + 1) * NH,
                                   NCH * c:NCH * (c + 1), :])
                        ft3 = ft.rearrange("p (c s) -> p c s", s=S)
                        red = wa.tile([NH, NCH], F32, name="redA", tag="redA")
                        nc.vector.reduce_sum(red[:], ft3, axis=AX)
                        col = NCH * c
                        nc.vector.tensor_add(
                            pa_s[:, col:col + NCH], pa_s[:, col:col + NCH], red[:])
                        sqa = wa.tile([NH, CHK], F32, name="sqA", tag="sqA")
                        nc.scalar.activation(sqa[:], ft[:], ACTF.Square)
                        red2 = wa.tile([NH, NCH], F32, name="redA2", tag="redA2")
                        nc.vector.reduce_sum(
                            red2[:], sqa.rearrange("p (c s) -> p c s", s=S), axis=AX)
                        nc.vector.tensor_add(
                            pa_s[:, C3 + col:C3 + col + NCH],
                            pa_s[:, C3 + col:C3 + col + NCH], red2[:])

                # partition-reduce: transpose [NH, 96] -> [96, NH], then free reduce
                pst = psT.tile([2 * C3, NH], F32)
                nc.tensor.transpose(pst[:], pa_s[:], eye_s[0:NH, 0:NH])
                sA = pp.tile([CP, 1], F32, name="sA")
                nc.vector.reduce_sum(sA[:], pst[:], axis=AX)

            sA_post = allreduce(sA, "bn1a")
            sc1a, bi1a = bn_scale_bias(sA_post, True, G1A, B1A, "bn1a")

            # ---------- segments B..E ----------
            with tc.tile_pool(name="psC", bufs=2, space="PSUM") as psC, \
                 tc.tile_pool(name="work", bufs=3) as wk:

                def load_x_parts(dst, v):
                    """Load [feats|pp|pn] for pair v into flat [CP, S] f32 tile."""
                    for n in range(2):
                        node = 2 * v + n
                        po = C3 * n
                        nc.sync.dma_start(dst[po:po + C, :], fown_d[node])
                        nc.sync.dma_start(dst[po + C:po + 2 * C, :], pooled_d[0, node])
                        nc.sync.dma_start(dst[po + 2 * C:po + 3 * C, :], pooled_d[1, node])

                # ======== segment B: h1 = conv1a(relu(bn1a(x))) + b1a ========
                for v in range(NP):
                    xp = xp_ring[v % N_XP]
                    xp3 = xp[:, 0:FLAT].rearrange("p (a b) -> p a b", b=W_PAD)
                    xbf = wk.tile([CP, S], F32, name="xbf", tag="xin", bufs=5)
                    load_x_parts(xbf, v)
                    nc.scalar.activation(xp3[:, 1:33, 1:33],
                                         xbf[:].rearrange("p (a b) -> p a b", b=32),
                                         ACTF.Relu, bias=bi1a[:], scale=sc1a[:])
                    ps = psC.tile([CP, PSW], F32, name="cps", tag="cps")
                    conv_mms(ps, w_s["1a"], CP, xp[:])
                    ps_int = ps.rearrange("p (r q) -> p r q", q=W_PAD)[:, :, 0:32]
                    h1t = wk.tile([CP, S], F32, name="h1t", tag="hseg")
                    nc.vector.tensor_scalar(
                        h1t[:].rearrange("p (r q) -> p r q", q=32), ps_int,
                        bnp_s[:, CB1A:CB1A + 1], 0.0, ALU.add, ALU.add,
                        accum_out=stw["B"][0][:, v:v + 1])
                    sqt = wk.tile([CP, S], F32, name="sqt", tag="sqseg")
                    nc.vector.scalar_tensor_tensor(
                        sqt[:], h1t[:], 1.0, h1t[:], ALU.mult, ALU.mult,
                        accum_out=stw["B"][1][:, v:v + 1])
                    nc.gpsimd.dma_start(h1_d[2 * v:2 * v + 2], h1t[:])

                svB = pp.tile([CP, 2], F32, name="svB")
                nc.vector.reduce_sum(svB[:, 0:1], stw["B"][0][:], axis=AX)
                nc.vector.reduce_sum(svB[:, 1:2], stw["B"][1][:], axis=AX)
                svB_post = allreduce(svB, "bn1b")
                sc1b, bi1b = bn_scale_bias(svB_post, False, G1B, B1B, "bn1b")

                # ======== segment C: x1 = x + conv1b(relu(bn1b(h1))) + b1b ========
                for v in range(NP):
                    xp = xp_ring[v % N_XP]
                    xp3 = xp[:, 0:FLAT].rearrange("p (a b) -> p a b", b=W_PAD)
                    hfl = wk.tile([CP, S], F32, name="hfl", tag="xin", bufs=5)
                    nc.sync.dma_start(hfl[:], h1_d[2 * v:2 * v + 2])
                    nc.scalar.activation(xp3[:, 1:33, 1:33],
                                         hfl[:].rearrange("p (a b) -> p a b", b=32),
                                         ACTF.Relu, bias=bi1b[:], scale=sc1b[:])
                    ps = psC.tile([CP, PSW], F32, name="cps", tag="cps")
                    conv_mms(ps, w_s["1b"], CP, xp[:])
                    xf = wk.tile([CP, S], F32, name="xf", tag="xfseg", bufs=5)
                    load_x_parts(xf, v)
                    ps_int = ps.rearrange("p (r q) -> p r q", q=W_PAD)[:, :, 0:32]
                    x1t = wk.tile([CP, S], F32, name="x1t", tag="hseg")
                    nc.vector.scalar_tensor_tensor(
                        x1t[:].rearrange("p (r q) -> p r q", q=32), ps_int,
                        bnp_s[:, CB1B:CB1B + 1],
                        xf[:].rearrange("p (r q) -> p r q", q=32),
                        ALU.add, ALU.add,
                        accum_out=stw["C"][0][:, v:v + 1])
                    sqt = wk.tile([CP, S], F32, name="sqt", tag="sqseg")
                    nc.vector.scalar_tensor_tensor(
                        sqt[:], x1t[:], 1.0, x1t[:], ALU.mult, ALU.mult,
                        accum_out=stw["C"][1][:, v:v + 1])
                    nc.gpsimd.dma_start(x1_d[2 * v:2 * v + 2], x1t[:])

                svC = pp.tile([CP, 2], F32, name="svC")
                nc.vector.reduce_sum(svC[:, 0:1], stw["C"][0][:], axis=AX)
                nc.vector.reduce_sum(svC[:, 1:2], stw["C"][1][:], axis=AX)
                svC_post = allreduce(svC, "bn2a")
                sc2a, bi2a = bn_scale_bias(svC_post, False, G2A, B2A, "bn2a")

                # ======== segment D: h3 = conv2a(relu(bn2a(x1))) + b2a ========
                for v in range(NP):
                    xp = xp_ring[v % N_XP]
                    xp3 = xp[:, 0:FLAT].rearrange("p (a b) -> p a b", b=W_PAD)
                    hfl = wk.tile([CP, S], F32, name="hfl", tag="xin", bufs=5)
                    nc.sync.dma_start(hfl[:], x1_d[2 * v:2 * v + 2])
                    nc.scalar.activation(xp3[:, 1:33, 1:33],
                                         hfl[:].rearrange("p (a b) -> p a b", b=32),
                                         ACTF.Relu, bias=bi2a[:], scale=sc2a[:])
                    ps = psC.tile([CP, PSW], F32, name="cps", tag="cps")
                    conv_mms(ps, w_s["2a"], CP, xp[:])
                    ps_int = ps.rearrange("p (r q) -> p r q", q=W_PAD)[:, :, 0:32]
                    h3t = wk.tile([CP, S], F32, name="h3t", tag="hseg")
                    nc.vector.tensor_scalar(
                        h3t[:].rearrange("p (r q) -> p r q", q=32), ps_int,
                        bnp_s[:, CB2A:CB2A + 1], 0.0, ALU.add, ALU.add,
                        accum_out=stw["D"][0][:, v:v + 1])
                    sqt = wk.tile([CP, S], F32, name="sqt", tag="sqseg")
                    nc.vector.scalar_tensor_tensor(
                        sqt[:], h3t[:], 1.0, h3t[:], ALU.mult, ALU.mult,
                        accum_out=stw["D"][1][:, v:v + 1])
                    nc.gpsimd.dma_start(h3_d[2 * v:2 * v + 2], h3t[:])

                svD = pp.tile([CP, 2], F32, name="svD")
                nc.vector.reduce_sum(svD[:, 0:1], stw["D"][0][:], axis=AX)
                nc.vector.reduce_sum(svD[:, 1:2], stw["D"][1][:], axis=AX)
                svD_post = allreduce(svD, "bn2b")
                sc2b, bi2b = bn_scale_bias(svD_post, False, G2B, B2B, "bn2b")

                # ======== segment E: x2 = x1 + conv2b(relu(bn2b(h3))) + b2b;
                #          out = relu(instnorm(convf(x2))) ========
                inv_s = 1.0 / S
                pend = None  # (y_sb, yr, ynb, v) awaiting final relu+store
                def flush_pend():
                    nonlocal pend
                    if pend is None:
                        return
                    p_ysb, p_yr, p_ynb, pv = pend
                    yo = wk.tile([2 * C, S], F32, name="yo", tag="yo")
                    nc.scalar.activation(yo[:], p_ysb[:], ACTF.Relu,
                                         bias=p_ynb[:], scale=p_yr[:])
                    nc.gpsimd.dma_start(out_d[2 * pv:2 * pv + 2], yo[:])
                    pend = None
                for v in range(NP):
                    xp = xp_ring[(2 * v) % N_XP]
                    xp3 = xp[:, 0:FLAT].rearrange("p (a b) -> p a b", b=W_PAD)
                    hfl = wk.tile([CP, S], F32, name="hfl", tag="xin", bufs=5)
                    nc.sync.dma_start(hfl[:], h3_d[2 * v:2 * v + 2])
                    nc.scalar.activation(xp3[:, 1:33, 1:33],
                                         hfl[:].rearrange("p (a b) -> p a b", b=32),
                                         ACTF.Relu, bias=bi2b[:], scale=sc2b[:])
                    flush_pend()
                    ps = psC.tile([CP, PSW], F32, name="cps", tag="cps")
                    conv_mms(ps, w_s["2b"], CP, xp[:])
                    x1f = wk.tile([CP, S], F32, name="x1f", tag="xfseg", bufs=5)
                    nc.sync.dma_start(x1f[:], x1_d[2 * v:2 * v + 2])
                    # x2 into padded tile (f32r) for convf
                    xq = xp_ring[(2 * v + 1) % N_XP]
                    xq3 = xq[:, 0:FLAT].rearrange("p (a b) -> p a b", b=W_PAD)
                    ps_int = ps.rearrange("p (r q) -> p r q", q=W_PAD)[:, :, 0:32]
                    nc.vector.scalar_tensor_tensor(
                        xq3[:, 1:33, 1:33], ps_int,
                        bnp_s[:, CB2B:CB2B + 1],
                        x1f[:].rearrange("p (r q) -> p r q", q=32),
                        ALU.add, ALU.add)
                    psy = psC.tile([2 * C, PSW], F32, name="cpsy", tag="cps")
                    conv_mms(psy, wf_s, 2 * C, xq[:])
                    # instance norm per (node, channel) partition
                    py_int = psy.rearrange("p (r q) -> p r q", q=W_PAD)[:, :, 0:32]
                    ysum = wk.tile([2 * C, 1], F32, name="ysum", tag="ysum")
                    y_sb = wk.tile([2 * C, S], F32, name="y_sb", tag="y_sb")
                    nc.vector.tensor_scalar(
                        y_sb[:].rearrange("p (r q) -> p r q", q=32), py_int,
                        0.0, 0.0, ALU.add, ALU.add, accum_out=ysum[:])
                    ysq = wk.tile([2 * C, S], F32, name="ysq", tag="ysq")
                    ysqs = wk.tile([2 * C, 1], F32, name="ysqs", tag="ysqs")
                    nc.vector.scalar_tensor_tensor(
                        ysq[:], y_sb[:], 1.0, y_sb[:], ALU.mult, ALU.mult,
                        accum_out=ysqs[:])
                    ym = wk.tile([2 * C, 1], F32, name="ym", tag="ym")
                    nc.vector.tensor_scalar_mul(ym[:], ysum[:], inv_s)
                    yv = wk.tile([2 * C, 1], F32, name="yv", tag="yv")
                    # var = ysqs/S - ym^2 = (ym * -ym) + ysqs/S
                    nc.vector.tensor_scalar_mul(yv[:], ysqs[:], inv_s)
                    ym2 = wk.tile([2 * C, 1], F32, name="ym2", tag="ym2")
                    nc.vector.tensor_mul(ym2[:], ym[:], ym[:])
                    nc.vector.tensor_sub(yv[:], yv[:], ym2[:])
                    nc.vector.tensor_scalar_add(yv[:], yv[:], EPS)
                    yrc = wk.tile([2 * C, 1], F32, name="yrc", tag="yrc")
                    nc.vector.reciprocal(yrc[:], yv[:])
                    yr = wk.tile([2 * C, 1], F32, name="yr", tag="yr")
                    nc.scalar.activation(yr[:], yrc[:], ACTF.Sqrt)
                    ynb = wk.tile([2 * C, 1], F32, name="ynb", tag="ynb")
                    nc.vector.tensor_mul(ynb[:], ym[:], yr[:])
                    nc.vector.tensor_scalar_mul(ynb[:], ynb[:], -1.0)
                    pend = (y_sb, yr, ynb, v)
                if pend is not None:
                    flush_pend()

    nc.compile()
    _PROGRAM_CACHE[key] = (nc, d)
    return nc, d


def _host_prep(feats, edges, params, cfg):
    d = _derived(cfg)
    V, C, S, NB = d["V"], d["C"], d["S"], d["NB"]
    Vc, NH, C3, CP, CHW, NG = d["Vc"], d["NH"], d["C3"], d["CP"], d["CHW"], d["NG"]
    NGB = NG * NB

    feats = np.asarray(feats, np.float32).reshape(V, CHW)
    edges = np.asarray(edges)
    src, sgn, dst = edges[:, 0], edges[:, 1], edges[:, 2]
    dsrc = np.concatenate([src, dst]).astype(np.int64)
    ddst = np.concatenate([dst, src]).astype(np.int64)
    dsgn = np.concatenate([sgn, sgn])

    # per-core edge groups
    idx_all = np.full((N_CORES, 128, NGB), 60000, np.uint32)
    smat_all = np.zeros((N_CORES, 128, NGB * NH), np.float32)
    for k in range(N_CORES):
        lo = k * Vc
        m = (ddst >= lo) & (ddst < lo + Vc)
        es, ed, eg = dsrc[m], ddst[m] - lo, dsgn[m]
        for g in range(NG):
            sign, half = g // 2, g % 2
            sel = ((eg < 0) if sign else (eg > 0)) & (ed // NH == half)
            gs, gd = es[sel], ed[sel] % NH
            order = np.argsort(gd, kind="stable")
            gs, gd = gs[order], gd[order]
            ne = len(gs)
            assert ne <= NB * 128, f"core {k} group {g}: {ne} edges > {NB*128}"
            for b in range(NB):
                gb = g * NB + b
                seg = slice(b * 128, min((b + 1) * 128, ne))
                n = seg.stop - seg.start
                if n <= 0:
                    continue
                idx_all[k, :n, gb] = gs[seg]
                smat_all[k, np.arange(n), gb * NH + gd[seg]] = 1.0

    # weights: block-diag per tap
    def pack_w(wname, cout):
        W = np.asarray(params[wname], np.float32)
        out = np.zeros((CP, 9 * 2 * cout), np.float32)
        for t in range(9):
            di, dj = t // 3, t % 3
            blk = W[:, :, di, dj].T  # [Cin=48, cout]
            out[0:C3, 2 * cout * t: 2 * cout * t + cout] = blk
            out[C3:CP, 2 * cout * t + cout: 2 * cout * (t + 1)] = blk
        return out

    w_host = {k: pack_w(f"conv{k}_w", C3) for k in ("1a", "1b", "2a", "2b")}
    wf_host = pack_w("convf_w", C)

    bnp = np.zeros((CP, 12), np.float32)
    cols = [("bn1a_g", 0), ("bn1a_b", 1), ("bn1b_g", 2), ("bn1b_b", 3),
            ("bn2a_g", 4), ("bn2a_b", 5), ("bn2b_g", 6), ("bn2b_b", 7),
            ("conv1a_b", 8), ("conv1b_b", 9), ("conv2a_b", 10), ("conv2b_b", 11)]
    for name, cc in cols:
        val = np.asarray(params[name], np.float32)
        bnp[0:C3, cc] = val
        bnp[C3:CP, cc] = val

    zz = np.zeros((128, d["CHK"]), np.float32)
    eye = np.eye(128, dtype=np.float32)

    in_maps = []
    for k in range(N_CORES):
        in_maps.append({
            "feats": feats,
            "feats_own": feats[k * Vc:(k + 1) * Vc].reshape(Vc, C, S),
            "gidx": idx_all[k],
            "smat": smat_all[k],
            **{f"w{kk}": w_host[kk] for kk in w_host},
            "wf": wf_host,
            "bnp": bnp,
            "zz": zz,
            "eye": eye,
        })
    return in_maps


def run(feats, edges, params, cfg=None, trace=False):
    cfg = cfg or FULL_CFG
    d = _derived(cfg)
    nc, _ = _build_program(cfg)
    in_maps = _host_prep(feats, edges, params, cfg)
    res = bass_utils.run_bass_kernel_spmd(
        nc, in_maps, core_ids=list(range(N_CORES)), trace=trace)
    out = np.concatenate([res.results[k]["out"] for k in range(N_CORES)], axis=0)
    out = out.reshape(cfg["V"], cfg["C"], 32, 32).astype(np.float32)
    return out, res


def kernel(feats, edges, params):
    out, _ = run(feats, edges, params, FULL_CFG)
    return out
